# revision 19
# baseline (speedup 1.0000x reference)
"""Trainium2 Bass kernel for nn_ComplexCrossAttention.

Strategy (v2):
- Data-parallel over batch B=8 across 8 NeuronCores (one batch element each,
  no collectives).
- All matmul operands are bf16 (PSUM accumulation fp32): enables FWL so
  LDWEIGHTS overlaps matmuls, and halves weight DMA vs fp32.
- QKV projections stay in the stacked-real form Z=[re;im] with prestacked
  weights; the complex MLP uses the Gauss 3-multiplication trick
  (T1=Ar Wr, T2=Ai Wi, T3=(Ar+Ai)(Wr+Wi)) cutting c_fc/c_proj PE time 25%.
- Attention per head: transposed scores St[k,q], exp straight out of PSUM,
  key-axis softmax sums via ones-matmuls, 1/denom via reciprocal_approx_fast,
  normalization + V-bias + query-residual folded into the AV eviction.
  The V bias is deferred through softmax (attention rows sum to 1):
  obias_r = bvr - bvi, obias_i = bvr + bvi added at eviction.
- Activations/weights shipped in partition-major contiguous layouts so all
  big DMAs are linear.
"""

import sys

for _p in ("/opt/trn_rl_repo",):
    if _p not in sys.path:
        sys.path.insert(0, _p)

import numpy as np
import ml_dtypes

import concourse.bass as bass
import concourse.mybir as mybir
import concourse.tile as tile
from concourse import bacc
from concourse.bass import broadcast_tensor_aps
from concourse.bass_utils import run_bass_kernel_spmd

BF16 = mybir.dt.bfloat16
FP32 = mybir.dt.float32
AF = mybir.ActivationFunctionType
OP = mybir.AluOpType

B, S, D = 8, 512, 1024
NH, DH = 16, 64
HID = 4096
T = S
N_CORES = 8
D2 = 2 * D       # 2048 stacked features
KC_D = D2 // 128   # 16 contraction chunks of the model dim
MC_D = D2 // 128   # 16 chunks of the model dim
OC_H = HID // 128  # 32 out chunks of one MLP hidden component
KC_H = HID // 128  # 32 contraction chunks of one hidden component
EPS = 1e-5
NPBF = ml_dtypes.bfloat16


def _build_nc():
    nc = bacc.Bacc(None, target_bir_lowering=False, debug=False)

    zq_d = nc.dram_tensor("zq", [128, KC_D, T], BF16, kind="ExternalInput")
    zx_d = nc.dram_tensor("zx", [128, KC_D, T], BF16, kind="ExternalInput")
    wq_d = nc.dram_tensor("wq", [MC_D, 128, KC_D, 128], BF16, kind="ExternalInput")
    wk_d = nc.dram_tensor("wk", [MC_D, 128, KC_D, 128], BF16, kind="ExternalInput")
    wv_d = nc.dram_tensor("wv", [NH // 2, 128, KC_D, 256], BF16, kind="ExternalInput")
    wfc_d = nc.dram_tensor("wfc", [3, OC_H, 128, 8, 128], BF16, kind="ExternalInput")
    wpj_d = nc.dram_tensor("wpj", [3, 8, 128, KC_H, 128], BF16, kind="ExternalInput")
    bq_d = nc.dram_tensor("bq", [128, MC_D], FP32, kind="ExternalInput")
    bk_d = nc.dram_tensor("bk", [128, MC_D], FP32, kind="ExternalInput")
    ob_d = nc.dram_tensor("ob", [128, NH], FP32, kind="ExternalInput")
    bfc_d = nc.dram_tensor("bfc", [128, 2 * OC_H], FP32, kind="ExternalInput")
    bp_d = nc.dram_tensor("bp", [128, 16], FP32, kind="ExternalInput")
    lng_d = nc.dram_tensor("lng", [128, 48], FP32, kind="ExternalInput")
    lnb_d = nc.dram_tensor("lnb", [128, 48], FP32, kind="ExternalInput")
    y_d = nc.dram_tensor("y", [128, MC_D, T], FP32, kind="ExternalOutput")

    with tile.TileContext(nc) as tc:
        consts_cm = tc.tile_pool(name="consts", bufs=1)
        consts = consts_cm.__enter__()

        ones_b = consts.tile([128, 1], BF16)
        nc.vector.memset(ones_b[:], 1.0)
        eps_t = consts.tile([128, 1], FP32)
        nc.vector.memset(eps_t[:], EPS)
        bq_s = consts.tile([128, MC_D], FP32)
        nc.sync.dma_start(bq_s[:], bq_d[:])
        bk_s = consts.tile([128, MC_D], FP32)
        nc.sync.dma_start(bk_s[:], bk_d[:])
        ob_s = consts.tile([128, NH], FP32)
        nc.sync.dma_start(ob_s[:], ob_d[:])
        bfc_s = consts.tile([128, 2 * OC_H], FP32)
        nc.sync.dma_start(bfc_s[:], bfc_d[:])
        bp_s = consts.tile([128, 16], FP32)
        nc.sync.dma_start(bp_s[:], bp_d[:])
        lng_s = consts.tile([128, 48], FP32)
        nc.sync.dma_start(lng_s[:], lng_d[:])
        lnb_s = consts.tile([128, 48], FP32)
        nc.sync.dma_start(lnb_s[:], lnb_d[:])

        def ln_gb(idx, comp, c8):
            j = idx * 16 + comp * 8 + c8
            return lng_s[:, j:j + 1], lnb_s[:, j:j + 1]

        # ---- long-lived activation pools (manually scoped, LIFO order:
        # entered in reverse order of release) ----
        yp_cm = tc.tile_pool(name="yp", bufs=1)
        yp_pool = yp_cm.__enter__()
        y_pre = yp_pool.tile([128, MC_D, T], BF16, name="y_pre")

        x2n_cm = tc.tile_pool(name="x2n", bufs=1)
        x2n_pool = x2n_cm.__enter__()
        x2n = x2n_pool.tile([128, MC_D, T], BF16, name="x2n")

        zx_cm = tc.tile_pool(name="zx", bufs=1)
        zx_pool = zx_cm.__enter__()
        zx_s = zx_pool.tile([128, KC_D, T], BF16, name="zx_s")

        zq_cm = tc.tile_pool(name="zq", bufs=1)
        zq_pool = zq_cm.__enter__()
        zq_s = zq_pool.tile([128, KC_D, T], BF16, name="zq_s")
        for i in range(4):
            nc.sync.dma_start(
                zq_s[:, i * 4:(i + 1) * 4, :], zq_d[:, i * 4:(i + 1) * 4, :]
            )

        o_cm = tc.tile_pool(name="op", bufs=1)
        o_pool = o_cm.__enter__()
        o_s = o_pool.tile([128, MC_D, T], BF16, name="o_s")

        q_cm = tc.tile_pool(name="qp", bufs=1)
        q_pool = q_cm.__enter__()
        q_s = q_pool.tile([128, NH, T], BF16, name="q_s")

        # =============== Phase A: Q projection (feature-major) ===============
        with (
            tc.tile_pool(name="wqp", bufs=3) as wq_pool,
            tc.tile_pool(name="psA", bufs=4, space="PSUM") as psA,
        ):
            for mc in range(MC_D):
                wt = wq_pool.tile([128, KC_D, 128], BF16, tag="wq")
                nc.sync.dma_start(wt[:], wq_d[mc])
                if mc == 1:
                    # x load deferred behind the first Q-proj weights so the
                    # PE isn't starved at kernel start (x isn't needed until
                    # the K/V projections in phase B).
                    for i in range(4):
                        nc.sync.dma_start(
                            zx_s[:, i * 4:(i + 1) * 4, :],
                            zx_d[:, i * 4:(i + 1) * 4, :],
                        )
                ps = psA.tile([128, T], FP32, tag="psA")
                for kc in range(KC_D):
                    nc.tensor.matmul(
                        ps[:], wt[:, kc, :], zq_s[:, kc, :],
                        start=(kc == 0), stop=(kc == KC_D - 1),
                    )
                nc.scalar.activation(
                    q_s[:, mc, :], ps[:], AF.Identity, bias=bq_s[:, mc:mc + 1]
                )

        # =============== Phase B: attention, head-streamed ===============
        with (
            tc.tile_pool(name="wkp", bufs=2) as wk_pool,
            tc.tile_pool(name="wvp", bufs=2) as wv_pool,
            tc.tile_pool(name="kp", bufs=4) as k_pool,
            tc.tile_pool(name="vp", bufs=2) as v_pool,
            tc.tile_pool(name="ep", bufs=10) as e_pool,
            tc.tile_pool(name="ttp", bufs=4) as tt_pool,
            tc.tile_pool(name="stp", bufs=2) as st_pool,
            tc.tile_pool(name="recp", bufs=4) as rec_pool,
            tc.tile_pool(name="bcp", bufs=4) as bc_pool,
            tc.tile_pool(name="psK", bufs=1, space="PSUM") as psK,
            tc.tile_pool(name="psV", bufs=1, space="PSUM") as psV,
            tc.tile_pool(name="psS", bufs=2, space="PSUM") as psS,
            tc.tile_pool(name="psO", bufs=2, space="PSUM") as psO,
            tc.tile_pool(name="psD", bufs=2, space="PSUM") as psD,
        ):
            v_cur = None
            for h in range(NH):
                hp, par = divmod(h, 2)
                if par == 0:
                    # V projection for the head pair (token-major), no bias
                    # (deferred through softmax into obias at eviction).
                    wvt = wv_pool.tile([128, KC_D, 256], BF16, tag="wv")
                    nc.sync.dma_start(wvt[:], wv_d[hp])
                    v_cur = v_pool.tile([128, 4, 512], BF16, tag="v")
                    for tcb in range(4):
                        psv = psV.tile([128, 256], FP32, tag="psV")
                        for kc in range(KC_D):
                            nc.tensor.matmul(
                                psv[:],
                                zx_s[:, kc, tcb * 128:(tcb + 1) * 128],
                                wvt[:, kc, :],
                                start=(kc == 0), stop=(kc == KC_D - 1),
                            )
                        for sub in range(2):
                            base = sub * 256
                            # V1 = [Vr | Vi]
                            nc.vector.tensor_copy(
                                v_cur[:, tcb, base:base + 128],
                                psv[:, sub * 128:(sub + 1) * 128],
                            )
                            # V2 = [-Vi | Vr]
                            nc.scalar.activation(
                                v_cur[:, tcb, base + 128:base + 192],
                                psv[:, sub * 128 + 64:sub * 128 + 128],
                                AF.Identity, scale=-1.0,
                            )
                            nc.scalar.activation(
                                v_cur[:, tcb, base + 192:base + 256],
                                psv[:, sub * 128:sub * 128 + 64],
                                AF.Copy,
                            )

                # K1 = [Kr; -Ki] projection (feature-major); K2 = [Ki; Kr]
                wkt = wk_pool.tile([128, KC_D, 128], BF16, tag="wk")
                nc.sync.dma_start(wkt[:], wk_d[h])
                k1 = k_pool.tile([128, T], BF16, tag="k")
                ps = psK.tile([128, T], FP32, tag="psK")
                for kc in range(KC_D):
                    nc.tensor.matmul(
                        ps[:], wkt[:, kc, :], zx_s[:, kc, :],
                        start=(kc == 0), stop=(kc == KC_D - 1),
                    )
                nc.scalar.activation(
                    k1[:], ps[:], AF.Identity, bias=bk_s[:, h:h + 1]
                )
                k2 = k_pool.tile([128, T], BF16, tag="k")
                nc.sync.dma_start(k2[0:64, :], k1[64:128, :])
                nc.vector.tensor_scalar_mul(k2[0:64, :], k2[0:64, :], -1.0)
                nc.sync.dma_start(k2[64:128, :], k1[0:64, :])
                k_t = [k1, k2]

                # transposed scores + exp (comp 0: re via K1, comp 1: im via K2)
                e_tiles = [[None] * 4 for _ in range(2)]
                for comp in range(2):
                    for kc4 in range(4):
                        pss = psS.tile([128, T], FP32, tag="psS")
                        nc.tensor.matmul(
                            pss[:],
                            k_t[comp][:, kc4 * 128:(kc4 + 1) * 128],
                            q_s[:, h, :],
                            start=True, stop=True,
                        )
                        et = e_pool.tile([128, T], BF16, tag="e")
                        nc.scalar.activation(et[:], pss[:], AF.Exp)
                        e_tiles[comp][kc4] = et

                # softmax denominators -> fast reciprocal -> broadcast
                bc = []
                for comp in range(2):
                    psd = psD.tile([1, T], FP32, tag="psD")
                    for kc4 in range(4):
                        nc.tensor.matmul(
                            psd[:], ones_b[:], e_tiles[comp][kc4],
                            start=(kc4 == 0), stop=(kc4 == 3),
                        )
                    rec = rec_pool.tile([1, T], FP32, tag="rec")
                    nc.vector.reciprocal_approx_fast(rec[:], psd[:])
                    bct = bc_pool.tile([128, T], FP32, tag="bc")
                    nc.gpsimd.partition_broadcast(bct[:], rec[:])
                    bc.append(bct)

                # AV: two accumulation groups (er-part needs /dr, ei-part /di)
                pso = []
                for comp in range(2):
                    p = psO.tile([128, T], FP32, tag="psO")
                    for kc4 in range(4):
                        base = par * 256 + comp * 128
                        nc.tensor.matmul(
                            p[:],
                            v_cur[:, kc4, base:base + 128],
                            e_tiles[comp][kc4],
                            start=(kc4 == 0), stop=(kc4 == 3),
                        )
                    pso.append(p)

                # eviction: comb = pso0/d_r + pso1/d_i + obias; rows
                # [Or(0:64); Oi(64:128)]; query residual fused here.
                c = h // 2
                ta = tt_pool.tile([128, T], FP32, tag="ta")
                tb = tt_pool.tile([128, T], FP32, tag="tb")
                comb = tt_pool.tile([128, T], BF16, tag="comb")
                nc.vector.tensor_tensor(ta[:], pso[0][:], bc[0][:], OP.mult)
                nc.vector.tensor_tensor(tb[:], pso[1][:], bc[1][:], OP.mult)
                nc.vector.scalar_tensor_tensor(
                    comb[:], ta[:], ob_s[:, h:h + 1], tb[:], OP.add, OP.add
                )
                if par == 0:
                    dsl, cc = slice(0, 64), c          # direct Or
                    ssl, sc = slice(64, 128), 8 + c    # staged Oi
                    msl = slice(0, 64)
                else:
                    dsl, cc = slice(64, 128), 8 + c    # direct Oi
                    ssl, sc = slice(0, 64), c          # staged Or
                    msl = slice(64, 128)
                nc.vector.tensor_tensor(
                    o_s[dsl, cc, :], comb[dsl, :], zq_s[dsl, cc, :], OP.add
                )
                stg = st_pool.tile([128, T], BF16, tag="stg")
                nc.sync.dma_start(stg[msl, :], comb[ssl, :])
                nc.vector.tensor_tensor(
                    o_s[msl, sc, :], stg[msl, :], zq_s[msl, sc, :], OP.add
                )

        q_cm.__exit__(None, None, None)

        # =============== LayerNorm helper ===============
        def btt(out_ap, in0_ap, in1_ap, op):
            """tensor_tensor with in1 broadcast along free dims of in0."""
            a, b2 = broadcast_tensor_aps(in0_ap, in1_ap)
            nc.vector.tensor_tensor(out_ap, a, b2, op)

        def layer_norm(src_t, dst_t, idx, psum_pool, small, bcast, sqp,
                       res_t=None, out_fp32=False, dma_out=None):
            """LN over the 1024 features of each of re (chunks 0-7) and im
            (chunks 8-15) of a [128, 16, T] tile. If res_t is given,
            dst = res + LN(src). If dma_out is given, it is called per
            4-chunk group with (c0, group_tile) after the group is done."""
            ps_mean = []
            ps_sq = []
            for comp in range(2):
                pm = psum_pool.tile([1, T], FP32, tag="lnpm")
                for c8 in range(8):
                    nc.tensor.matmul(
                        pm[:], ones_b[:], src_t[:, comp * 8 + c8, :],
                        start=(c8 == 0), stop=(c8 == 7),
                    )
                ps_mean.append(pm)
                pq = psum_pool.tile([1, T], FP32, tag="lnpq")
                for g4 in range(2):
                    sq = sqp.tile([128, 4, T], BF16, tag="sq")
                    nc.scalar.activation(
                        sq[:], src_t[:, comp * 8 + g4 * 4:comp * 8 + g4 * 4 + 4, :],
                        AF.Square,
                    )
                    for j in range(4):
                        nc.tensor.matmul(
                            pq[:], ones_b[:], sq[:, j, :],
                            start=(g4 == 0 and j == 0), stop=(g4 == 1 and j == 3),
                        )
                ps_sq.append(pq)
            bcs = []
            for comp in range(2):
                mean = small.tile([1, T], FP32, tag="mean")
                nc.scalar.activation(
                    mean[:], ps_mean[comp][:], AF.Identity, scale=1.0 / D
                )
                m2 = small.tile([1, T], FP32, tag="m2")
                nc.scalar.activation(m2[:], mean[:], AF.Square)
                var = small.tile([1, T], FP32, tag="var")
                nc.vector.scalar_tensor_tensor(
                    var[:], ps_sq[comp][:], 1.0 / D, m2[:], OP.mult, OP.subtract
                )
                sstd = small.tile([1, T], FP32, tag="sstd")
                nc.scalar.activation(sstd[:], var[:], AF.Sqrt, bias=eps_t[0:1, :])
                rstd = small.tile([1, T], FP32, tag="rstd")
                nc.vector.reciprocal_approx_fast(rstd[:], sstd[:])
                mr = small.tile([1, T], FP32, tag="mr")
                nc.vector.tensor_tensor(mr[:], mean[:], rstd[:], OP.mult)
                br = bcast.tile([128, 1, T], FP32, tag="br")
                nc.gpsimd.partition_broadcast(br[:, 0, :], rstd[:])
                bm = bcast.tile([128, 1, T], FP32, tag="bm")
                nc.gpsimd.partition_broadcast(bm[:, 0, :], mr[:])
                bcs.append((br, bm))
            out_dt = FP32 if out_fp32 else BF16
            for comp in range(2):
                br, bm = bcs[comp]
                for g4 in range(2):
                    c0 = comp * 8 + g4 * 4
                    t1 = sqp.tile([128, 4, T], BF16, tag="lnt1")
                    btt(t1[:], src_t[:, c0:c0 + 4, :], br[:], OP.mult)
                    vh = sqp.tile([128, 4, T], BF16, tag="lnvh")
                    btt(vh[:], t1[:], bm[:], OP.subtract)
                    # affine (·g + b): Act for comp 0, DVE for comp 1
                    if res_t is None and dma_out is None:
                        aff = None
                        aff_dst = lambda j: dst_t[:, c0 + j, :]
                    else:
                        aff = sqp.tile([128, 4, T], out_dt, tag="lnaf")
                        aff_dst = lambda j: aff[:, j, :]
                    for j in range(4):
                        g_ap, b_ap = ln_gb(idx, comp, g4 * 4 + j)
                        if comp == 0:
                            nc.scalar.activation(
                                aff_dst(j), vh[:, j, :], AF.Identity,
                                bias=b_ap, scale=g_ap,
                            )
                        else:
                            nc.vector.tensor_scalar(
                                aff_dst(j), vh[:, j, :], g_ap, b_ap,
                                OP.mult, OP.add,
                            )
                    if res_t is not None:
                        nc.vector.tensor_tensor(
                            dst_t[:, c0:c0 + 4, :], aff[:],
                            res_t[:, c0:c0 + 4, :], OP.add,
                        )
                    elif dma_out is not None:
                        dma_out(c0, aff)

        # =============== Phase C: two layernorms ===============
        with (
            tc.tile_pool(name="lnsq", bufs=4) as sq_pool,
            tc.tile_pool(name="lnsm", bufs=1) as small_pool,
            tc.tile_pool(name="lnbc", bufs=4) as bc2_pool,
            tc.tile_pool(name="psC", bufs=4, space="PSUM") as psC,
        ):
            # LN#0 over (attn_out + query) [already fused], + x residual,
            # written into zx_s (x2pre)
            layer_norm(
                o_s, zx_s, 0, psC, small_pool, bc2_pool, sq_pool, res_t=zx_s,
            )
            # LN#1 over x2pre -> x2n
            layer_norm(
                zx_s, x2n, 1, psC, small_pool, bc2_pool, sq_pool,
            )

        o_cm.__exit__(None, None, None)
        zq_cm.__exit__(None, None, None)
        zx_cm.__exit__(None, None, None)

        # =============== Phase D: complex MLP (Gauss 3-mult) ===============
        with (
            tc.tile_pool(name="xsump", bufs=1) as xsum_pool,
            tc.tile_pool(name="hp", bufs=1) as h_pool,
            tc.tile_pool(name="wfcp", bufs=6) as wfc_pool,
            tc.tile_pool(name="wpjp", bufs=3) as wpj_pool,
            tc.tile_pool(name="mrt", bufs=2) as mr_pool,
            tc.tile_pool(name="psF", bufs=6, space="PSUM") as psF,
        ):
            xsum = xsum_pool.tile([128, 8, T], BF16, name="xsum")
            for c8 in range(8):
                nc.vector.tensor_tensor(
                    xsum[:, c8, :], x2n[:, c8, :], x2n[:, 8 + c8, :], OP.add
                )

            hr_t = h_pool.tile([128, OC_H, T], BF16, name="hr")
            hi_t = h_pool.tile([128, OC_H, T], BF16, name="hi")
            hs_t = h_pool.tile([128, OC_H, T], BF16, name="hs")

            # c_fc: per out chunk, three Gauss matmul groups
            for oc in range(OC_H):
                wts = []
                pss = []
                for g in range(3):
                    wt = wfc_pool.tile([128, 8, 128], BF16, tag="wfc")
                    nc.sync.dma_start(wt[:], wfc_d[g, oc])
                    wts.append(wt)
                    p = psF.tile([128, T], FP32, tag="psF")
                    src_base = (0, 8, 0)[g]
                    src = x2n if g < 2 else xsum
                    for kc in range(8):
                        nc.tensor.matmul(
                            p[:], wt[:, kc, :],
                            (src[:, src_base + kc, :] if g < 2
                             else xsum[:, kc, :]),
                            start=(kc == 0), stop=(kc == 7),
                        )
                    pss.append(p)
                # Hr = (T1 + br) - T2 ; Hi = ((T3 + bi) - T1) - T2
                # (DVE reads at most one PSUM operand: evict T1 via Act first)
                t1sb = mr_pool.tile([128, T], FP32, tag="t1sb")
                nc.scalar.activation(t1sb[:], pss[0][:], AF.Copy)
                nc.vector.scalar_tensor_tensor(
                    hr_t[:, oc, :], t1sb[:], bfc_s[:, oc:oc + 1], pss[1][:],
                    OP.add, OP.subtract,
                )
                tmp = mr_pool.tile([128, T], FP32, tag="gtmp")
                nc.vector.scalar_tensor_tensor(
                    tmp[:], pss[2][:], bfc_s[:, OC_H + oc:OC_H + oc + 1],
                    t1sb[:], OP.add, OP.subtract,
                )
                nc.vector.tensor_tensor(
                    hi_t[:, oc, :], tmp[:], pss[1][:], OP.subtract
                )
                # modReLU: hr += |h| (0.5 folded into wpj); hs = hr' + hi
                sq1 = mr_pool.tile([128, T], FP32, tag="mr1")
                nc.scalar.activation(sq1[:], hr_t[:, oc, :], AF.Square)
                sq2 = mr_pool.tile([128, T], FP32, tag="mr2")
                nc.scalar.activation(sq2[:], hi_t[:, oc, :], AF.Square)
                nc.vector.tensor_tensor(sq1[:], sq1[:], sq2[:], OP.add)
                mag = mr_pool.tile([128, T], BF16, tag="mag")
                nc.scalar.activation(mag[:], sq1[:], AF.Sqrt)
                nc.vector.tensor_tensor(
                    hr_t[:, oc, :], hr_t[:, oc, :], mag[:], OP.add
                )
                nc.vector.tensor_tensor(
                    hs_t[:, oc, :], hr_t[:, oc, :], hi_t[:, oc, :], OP.add
                )

            # c_proj: per out chunk pc, U1/U2/U3 Gauss groups; final
            # bias + x2n residual fused into eviction -> y_pre
            for pc in range(8):
                ups = []
                for g, hsrc in ((0, hr_t), (1, hi_t), (2, hs_t)):
                    wt = wpj_pool.tile([128, KC_H, 128], BF16, tag="wpj")
                    nc.sync.dma_start(wt[:], wpj_d[g, pc])
                    p = psF.tile([128, T], FP32, tag="psF")
                    for kc in range(KC_H):
                        nc.tensor.matmul(
                            p[:], wt[:, kc, :], hsrc[:, kc, :],
                            start=(kc == 0), stop=(kc == KC_H - 1),
                        )
                    ups.append(p)
                # Mr = (U1 + bpr) - U2 (+ x2n_r)
                u1sb = mr_pool.tile([128, T], FP32, tag="u1sb")
                nc.scalar.activation(u1sb[:], ups[0][:], AF.Copy)
                tmp = mr_pool.tile([128, T], BF16, tag="gtmp2")
                nc.vector.scalar_tensor_tensor(
                    tmp[:], u1sb[:], bp_s[:, pc:pc + 1], ups[1][:],
                    OP.add, OP.subtract,
                )
                nc.vector.tensor_tensor(
                    y_pre[:, pc, :], tmp[:], x2n[:, pc, :], OP.add
                )
                # Mi = ((U3 + bpi) - U1) - U2 (+ x2n_i)
                tmp2 = mr_pool.tile([128, T], FP32, tag="gtmp3")
                nc.vector.scalar_tensor_tensor(
                    tmp2[:], ups[2][:], bp_s[:, 8 + pc:8 + pc + 1], u1sb[:],
                    OP.add, OP.subtract,
                )
                tmp3 = mr_pool.tile([128, T], BF16, tag="gtmp4")
                nc.vector.tensor_tensor(tmp3[:], tmp2[:], ups[1][:], OP.subtract)
                nc.vector.tensor_tensor(
                    y_pre[:, 8 + pc, :], tmp3[:], x2n[:, 8 + pc, :], OP.add
                )

        x2n_cm.__exit__(None, None, None)

        # =============== final layernorm + store ===============
        with (
            tc.tile_pool(name="lnsq2", bufs=4) as sq2_pool,
            tc.tile_pool(name="lnsm2", bufs=1) as small2_pool,
            tc.tile_pool(name="lnbc2", bufs=4) as bc3_pool,
            tc.tile_pool(name="psC2", bufs=4, space="PSUM") as psC2,
        ):
            layer_norm(
                y_pre, None, 2, psC2, small2_pool, bc3_pool, sq2_pool,
                out_fp32=True,
                dma_out=lambda c0, aff: nc.sync.dma_start(
                    y_d[:, c0:c0 + 4, :], aff[:]
                ),
            )

        yp_cm.__exit__(None, None, None)
        consts_cm.__exit__(None, None, None)

    nc.compile()
    if not nc.is_finalized():
        nc.finalize()
    return nc


def _stackT(w):
    """[F, Din, 2] torch-layout complex weight -> [2*Din, 2*F] stacked lhsT."""
    wr = w[..., 0].astype(np.float32)
    wi = w[..., 1].astype(np.float32)
    top = np.concatenate([wr.T, wi.T], axis=1)
    bot = np.concatenate([-wi.T, wr.T], axis=1)
    return np.concatenate([top, bot], axis=0)


def _prep_weights(wq, bq, wk, bk, wv, bv, w_fc, b_fc, w_proj, b_proj, ln_g, ln_b):
    qcols = np.concatenate(
        [np.concatenate([np.arange(h * 64, h * 64 + 64),
                         1024 + np.arange(h * 64, h * 64 + 64)]) for h in range(NH)]
    )
    scale = np.float32(1.0 / np.sqrt(DH))

    sq = _stackT(wq) * scale
    wq_t = np.ascontiguousarray(
        sq[:, qcols].reshape(KC_D, 128, MC_D, 128).transpose(2, 1, 0, 3)
    ).astype(NPBF)
    bq_l = (np.concatenate([bq[:, 0], bq[:, 1]]) * scale)[qcols]
    bq_a = np.ascontiguousarray(
        bq_l.reshape(MC_D, 128).T.astype(np.float32)
    )

    sk = _stackT(wk)
    bkst = np.concatenate([bk[:, 0], bk[:, 1]]).astype(np.float32)
    wk_full = sk[:, qcols].copy()           # [2048, 2048]: per head [Kr | Ki]
    bk_l = bkst[qcols].copy()
    for h in range(NH):
        wk_full[:, h * 128 + 64:h * 128 + 128] *= -1.0   # -> [Kr | -Ki]
        bk_l[h * 128 + 64:h * 128 + 128] *= -1.0
    wk_t = np.ascontiguousarray(
        wk_full.reshape(KC_D, 128, MC_D, 128).transpose(2, 1, 0, 3)
    ).astype(NPBF)
    bk_a = np.ascontiguousarray(bk_l.reshape(MC_D, 128).T.astype(np.float32))

    sv = _stackT(wv)
    svq = sv[:, qcols]                       # [2048, 2048]
    wv_t = np.ascontiguousarray(
        svq.reshape(KC_D, 128, NH // 2, 256).transpose(2, 1, 0, 3)
    ).astype(NPBF)
    # obias: V bias deferred through softmax; per head column:
    # rows 0:64 = bvr - bvi (Or), rows 64:128 = bvr + bvi (Oi)
    ob = np.empty((128, NH), dtype=np.float32)
    bvr, bvi = bv[:, 0].astype(np.float32), bv[:, 1].astype(np.float32)
    for h in range(NH):
        sl = slice(h * 64, h * 64 + 64)
        ob[0:64, h] = bvr[sl] - bvi[sl]
        ob[64:128, h] = bvr[sl] + bvi[sl]

    # Gauss c_fc: blocks Wr^T, Wi^T, (Wr+Wi)^T  [1024, 4096]
    fr = w_fc[..., 0].astype(np.float32).T
    fi = w_fc[..., 1].astype(np.float32).T
    wfc_t = np.ascontiguousarray(
        np.stack([fr, fi, fr + fi])
        .reshape(3, 8, 128, OC_H, 128).transpose(0, 3, 2, 1, 4)
    ).astype(NPBF)
    bfc_a = np.ascontiguousarray(
        np.concatenate([b_fc[:, 0], b_fc[:, 1]])
        .reshape(2 * OC_H, 128).T.astype(np.float32)
    )

    # Gauss c_proj (0.5 of modReLU folded into weights): [4096, 1024] blocks
    pr = (w_proj[..., 0].astype(np.float32) * 0.5).T
    pi = (w_proj[..., 1].astype(np.float32) * 0.5).T
    wpj_t = np.ascontiguousarray(
        np.stack([pr, pi, pr + pi])
        .reshape(3, KC_H, 128, 8, 128).transpose(0, 3, 2, 1, 4)
    ).astype(NPBF)
    bp_a = np.ascontiguousarray(
        np.concatenate([b_proj[:, 0], b_proj[:, 1]])
        .reshape(16, 128).T.astype(np.float32)
    )

    lng_a = np.ascontiguousarray(
        ln_g.astype(np.float32).reshape(3, 2, 8, 128).transpose(3, 0, 1, 2).reshape(128, 48)
    )
    lnb_a = np.ascontiguousarray(
        ln_b.astype(np.float32).reshape(3, 2, 8, 128).transpose(3, 0, 1, 2).reshape(128, 48)
    )
    return {
        "wq": wq_t, "bq": bq_a, "wk": wk_t, "bk": bk_a, "wv": wv_t, "ob": ob,
        "wfc": wfc_t, "bfc": bfc_a, "wpj": wpj_t, "bp": bp_a,
        "lng": lng_a, "lnb": lnb_a,
    }


_NC_CACHE = {}


def kernel(**inputs):
    if "nc" not in _NC_CACHE:
        _NC_CACHE["nc"] = _build_nc()
    nc = _NC_CACHE["nc"]

    x = np.asarray(inputs["x"], dtype=np.float32)
    query = np.asarray(inputs["query"], dtype=np.float32)
    shared = _prep_weights(
        np.asarray(inputs["wq"]), np.asarray(inputs["bq"]),
        np.asarray(inputs["wk"]), np.asarray(inputs["bk"]),
        np.asarray(inputs["wv"]), np.asarray(inputs["bv"]),
        np.asarray(inputs["w_fc"]), np.asarray(inputs["b_fc"]),
        np.asarray(inputs["w_proj"]), np.asarray(inputs["b_proj"]),
        np.asarray(inputs["ln_g"]), np.asarray(inputs["ln_b"]),
    )

    in_maps = []
    for b in range(B):
        zq = np.ascontiguousarray(
            np.concatenate([query[b, :, :, 0].T, query[b, :, :, 1].T], axis=0)
            .reshape(KC_D, 128, T).transpose(1, 0, 2)
        ).astype(NPBF)
        zx = np.ascontiguousarray(
            np.concatenate([x[b, :, :, 0].T, x[b, :, :, 1].T], axis=0)
            .reshape(KC_D, 128, T).transpose(1, 0, 2)
        ).astype(NPBF)
        m = {"zq": zq, "zx": zx}
        m.update(shared)
        in_maps.append(m)

    import os
    trace = bool(os.environ.get("KERNEL_TRACE"))
    res = run_bass_kernel_spmd(nc, in_maps, list(range(N_CORES)), trace=trace)
    _NC_CACHE["exec_time_ns"] = res.exec_time_ns
    out = np.empty((B, S, D, 2), dtype=np.float32)
    for b in range(B):
        yb = res.results[b]["y"].transpose(1, 0, 2).reshape(D2, T)
        out[b, :, :, 0] = yb[:D, :].T
        out[b, :, :, 1] = yb[D:, :].T
    return out


if __name__ == "__main__":
    rng = np.random.default_rng(0)
    f = np.float32
    demo = {
        "x": rng.standard_normal((B, S, D, 2), dtype=f),
        "query": rng.standard_normal((B, S, D, 2), dtype=f),
        "wq": rng.standard_normal((D, D, 2), dtype=f) * 0.02,
        "bq": rng.standard_normal((D, 2), dtype=f) * 0.02,
        "wk": rng.standard_normal((D, D, 2), dtype=f) * 0.02,
        "bk": rng.standard_normal((D, 2), dtype=f) * 0.02,
        "wv": rng.standard_normal((D, D, 2), dtype=f) * 0.02,
        "bv": rng.standard_normal((D, 2), dtype=f) * 0.02,
        "w_fc": rng.standard_normal((HID, D, 2), dtype=f) * 0.02,
        "b_fc": rng.standard_normal((HID, 2), dtype=f) * 0.02,
        "w_proj": rng.standard_normal((D, HID, 2), dtype=f) * 0.02,
        "b_proj": rng.standard_normal((D, 2), dtype=f) * 0.02,
        "ln_g": np.ones((3, 2, D), dtype=f),
        "ln_b": np.zeros((3, 2, D), dtype=f),
    }
    out = kernel(**demo)
    print("out shape", out.shape)


# revision 27
# speedup vs baseline: 1.1202x; 1.1202x over previous
"""Trainium2 Bass kernel for nn_ComplexCrossAttention.

Strategy (v2):
- Data-parallel over batch B=8 across 8 NeuronCores (one batch element each,
  no collectives).
- All matmul operands are bf16 (PSUM accumulation fp32): enables FWL so
  LDWEIGHTS overlaps matmuls, and halves weight DMA vs fp32.
- QKV projections stay in the stacked-real form Z=[re;im] with prestacked
  weights; the complex MLP uses the Gauss 3-multiplication trick
  (T1=Ar Wr, T2=Ai Wi, T3=(Ar+Ai)(Wr+Wi)) cutting c_fc/c_proj PE time 25%.
- Attention per head: transposed scores St[k,q], exp straight out of PSUM,
  key-axis softmax sums via ones-matmuls, 1/denom via reciprocal_approx_fast,
  normalization + V-bias + query-residual folded into the AV eviction.
  The V bias is deferred through softmax (attention rows sum to 1):
  obias_r = bvr - bvi, obias_i = bvr + bvi added at eviction.
- Activations/weights shipped in partition-major contiguous layouts so all
  big DMAs are linear.
"""

import sys

for _p in ("/opt/trn_rl_repo",):
    if _p not in sys.path:
        sys.path.insert(0, _p)

import numpy as np
import ml_dtypes

import concourse.bass as bass
import concourse.mybir as mybir
import concourse.tile as tile
from concourse import bacc
from concourse.bass import broadcast_tensor_aps
from concourse.bass_utils import run_bass_kernel_spmd

BF16 = mybir.dt.bfloat16
FP32 = mybir.dt.float32
AF = mybir.ActivationFunctionType
OP = mybir.AluOpType

B, S, D = 8, 512, 1024
NH, DH = 16, 64
HID = 4096
T = S
N_CORES = 8
D2 = 2 * D       # 2048 stacked features
KC_D = D2 // 128   # 16 contraction chunks of the model dim
MC_D = D2 // 128   # 16 chunks of the model dim
OC_H = HID // 128  # 32 out chunks of one MLP hidden component
KC_H = HID // 128  # 32 contraction chunks of one hidden component
EPS = 1e-5
NPBF = ml_dtypes.bfloat16


def _build_nc():
    nc = bacc.Bacc(None, target_bir_lowering=False, debug=False)

    zq_d = nc.dram_tensor("zq", [128, KC_D, T], BF16, kind="ExternalInput")
    zx_d = nc.dram_tensor("zx", [128, KC_D, T], BF16, kind="ExternalInput")
    wq_d = nc.dram_tensor("wq", [MC_D, 128, KC_D, 128], BF16, kind="ExternalInput")
    wk_d = nc.dram_tensor("wk", [MC_D, 128, KC_D, 128], BF16, kind="ExternalInput")
    wv_d = nc.dram_tensor("wv", [NH // 2, 128, KC_D, 256], BF16, kind="ExternalInput")
    wfc_d = nc.dram_tensor("wfc", [3, OC_H, 128, 8, 128], BF16, kind="ExternalInput")
    wpj_d = nc.dram_tensor("wpj", [3, 8, 128, KC_H, 128], BF16, kind="ExternalInput")
    bq_d = nc.dram_tensor("bq", [128, MC_D], FP32, kind="ExternalInput")
    bk_d = nc.dram_tensor("bk", [128, MC_D], FP32, kind="ExternalInput")
    ob_d = nc.dram_tensor("ob", [128, NH], FP32, kind="ExternalInput")
    bfc_d = nc.dram_tensor("bfc", [128, 2 * OC_H], FP32, kind="ExternalInput")
    bp_d = nc.dram_tensor("bp", [128, 16], FP32, kind="ExternalInput")
    lng_d = nc.dram_tensor("lng", [128, 48], FP32, kind="ExternalInput")
    lnb_d = nc.dram_tensor("lnb", [128, 48], FP32, kind="ExternalInput")
    y_d = nc.dram_tensor("y", [128, MC_D, T], FP32, kind="ExternalOutput")

    with tile.TileContext(nc) as tc:
        consts_cm = tc.tile_pool(name="consts", bufs=1)
        consts = consts_cm.__enter__()

        ones_b = consts.tile([128, 1], BF16)
        nc.vector.memset(ones_b[:], 1.0)
        eps_t = consts.tile([128, 1], FP32)
        nc.vector.memset(eps_t[:], EPS)
        bq_s = consts.tile([128, MC_D], FP32)
        nc.sync.dma_start(bq_s[:], bq_d[:])
        bk_s = consts.tile([128, MC_D], FP32)
        nc.sync.dma_start(bk_s[:], bk_d[:])
        ob_s = consts.tile([128, NH], FP32)
        nc.sync.dma_start(ob_s[:], ob_d[:])
        bfc_s = consts.tile([128, 2 * OC_H], FP32)
        nc.sync.dma_start(bfc_s[:], bfc_d[:])
        bp_s = consts.tile([128, 16], FP32)
        nc.sync.dma_start(bp_s[:], bp_d[:])
        lng_s = consts.tile([128, 48], FP32)
        nc.sync.dma_start(lng_s[:], lng_d[:])
        lnb_s = consts.tile([128, 48], FP32)
        nc.sync.dma_start(lnb_s[:], lnb_d[:])

        def ln_gb(idx, comp, c8):
            j = idx * 16 + comp * 8 + c8
            return lng_s[:, j:j + 1], lnb_s[:, j:j + 1]

        # ---- long-lived activation pools (manually scoped, LIFO order:
        # entered in reverse order of release) ----
        yp_cm = tc.tile_pool(name="yp", bufs=1)
        yp_pool = yp_cm.__enter__()
        y_pre = yp_pool.tile([128, MC_D, T], BF16, name="y_pre")

        wfc_cm = tc.tile_pool(name="wfcp", bufs=6)
        wfc_pool = wfc_cm.__enter__()
        wpj_cm = tc.tile_pool(name="wpjp", bufs=2)
        wpj_pool = wpj_cm.__enter__()

        x2n_cm = tc.tile_pool(name="x2n", bufs=1)
        x2n_pool = x2n_cm.__enter__()
        x2n = x2n_pool.tile([128, MC_D, T], BF16, name="x2n")

        zx_cm = tc.tile_pool(name="zx", bufs=1)
        zx_pool = zx_cm.__enter__()
        zx_s = zx_pool.tile([128, KC_D, T], BF16, name="zx_s")

        zq_cm = tc.tile_pool(name="zq", bufs=1)
        zq_pool = zq_cm.__enter__()
        zq_s = zq_pool.tile([128, KC_D, T], BF16, name="zq_s")
        for i in range(4):
            nc.sync.dma_start(
                zq_s[:, i * 4:(i + 1) * 4, :], zq_d[:, i * 4:(i + 1) * 4, :]
            )

        o_cm = tc.tile_pool(name="op", bufs=1)
        o_pool = o_cm.__enter__()
        o_s = o_pool.tile([128, MC_D, T], BF16, name="o_s")

        q_cm = tc.tile_pool(name="qp", bufs=1)
        q_pool = q_cm.__enter__()
        q_s = q_pool.tile([128, NH, T], BF16, name="q_s")

        # =============== Phase A: Q projection (feature-major) ===============
        with (
            tc.tile_pool(name="wqp", bufs=3) as wq_pool,
            tc.tile_pool(name="psA", bufs=4, space="PSUM") as psA,
        ):
            for mc in range(MC_D):
                wt = wq_pool.tile([128, KC_D, 128], BF16, tag="wq")
                nc.sync.dma_start(wt[:], wq_d[mc])
                if mc == 1:
                    # x load deferred behind the first Q-proj weights so the
                    # PE isn't starved at kernel start (x isn't needed until
                    # the K/V projections in phase B).
                    for i in range(4):
                        nc.sync.dma_start(
                            zx_s[:, i * 4:(i + 1) * 4, :],
                            zx_d[:, i * 4:(i + 1) * 4, :],
                        )
                ps = psA.tile([128, T], FP32, tag="psA")
                for kc in range(KC_D):
                    nc.tensor.matmul(
                        ps[:], wt[:, kc, :], zq_s[:, kc, :],
                        start=(kc == 0), stop=(kc == KC_D - 1),
                    )
                nc.scalar.activation(
                    q_s[:, mc, :], ps[:], AF.Identity, bias=bq_s[:, mc:mc + 1]
                )

        # =============== Phase B: attention, head-streamed ===============
        with (
            tc.tile_pool(name="wkp", bufs=2) as wk_pool,
            tc.tile_pool(name="wvp", bufs=2) as wv_pool,
            tc.tile_pool(name="kp", bufs=4) as k_pool,
            tc.tile_pool(name="vp", bufs=2) as v_pool,
            tc.tile_pool(name="ep", bufs=10) as e_pool,
            tc.tile_pool(name="ttp", bufs=4) as tt_pool,
            tc.tile_pool(name="stp", bufs=2) as st_pool,
            tc.tile_pool(name="recp", bufs=4) as rec_pool,
            tc.tile_pool(name="bcp", bufs=3) as bc_pool,
            tc.tile_pool(name="psK", bufs=1, space="PSUM") as psK,
            tc.tile_pool(name="psV", bufs=1, space="PSUM") as psV,
            tc.tile_pool(name="psS", bufs=2, space="PSUM") as psS,
            tc.tile_pool(name="psO", bufs=2, space="PSUM") as psO,
            tc.tile_pool(name="psD", bufs=2, space="PSUM") as psD,
        ):
            v_cur = None
            for h in range(NH):
                hp, par = divmod(h, 2)
                if par == 0:
                    # V projection for the head pair (token-major), no bias
                    # (deferred through softmax into obias at eviction).
                    wvt = wv_pool.tile([128, KC_D, 256], BF16, tag="wv")
                    nc.sync.dma_start(wvt[:], wv_d[hp])
                    v_cur = v_pool.tile([128, 4, 512], BF16, tag="v")
                    for tcb in range(4):
                        psv = psV.tile([128, 256], FP32, tag="psV")
                        for kc in range(KC_D):
                            nc.tensor.matmul(
                                psv[:],
                                zx_s[:, kc, tcb * 128:(tcb + 1) * 128],
                                wvt[:, kc, :],
                                start=(kc == 0), stop=(kc == KC_D - 1),
                            )
                        for sub in range(2):
                            base = sub * 256
                            # V1 = [Vr | Vi]
                            nc.vector.tensor_copy(
                                v_cur[:, tcb, base:base + 128],
                                psv[:, sub * 128:(sub + 1) * 128],
                            )
                            # V2 = [-Vi | Vr]
                            nc.scalar.activation(
                                v_cur[:, tcb, base + 128:base + 192],
                                psv[:, sub * 128 + 64:sub * 128 + 128],
                                AF.Identity, scale=-1.0,
                            )
                            nc.scalar.activation(
                                v_cur[:, tcb, base + 192:base + 256],
                                psv[:, sub * 128:sub * 128 + 64],
                                AF.Copy,
                            )

                # K1 = [Kr; -Ki] projection (feature-major); K2 = [Ki; Kr]
                wkt = wk_pool.tile([128, KC_D, 128], BF16, tag="wk")
                nc.sync.dma_start(wkt[:], wk_d[h])
                k1 = k_pool.tile([128, T], BF16, tag="k")
                ps = psK.tile([128, T], FP32, tag="psK")
                for kc in range(KC_D):
                    nc.tensor.matmul(
                        ps[:], wkt[:, kc, :], zx_s[:, kc, :],
                        start=(kc == 0), stop=(kc == KC_D - 1),
                    )
                nc.scalar.activation(
                    k1[:], ps[:], AF.Identity, bias=bk_s[:, h:h + 1]
                )
                k2 = k_pool.tile([128, T], BF16, tag="k")
                nc.sync.dma_start(k2[0:64, :], k1[64:128, :])
                nc.vector.tensor_scalar_mul(k2[0:64, :], k2[0:64, :], -1.0)
                nc.sync.dma_start(k2[64:128, :], k1[0:64, :])
                k_t = [k1, k2]

                # transposed scores + exp (comp 0: re via K1, comp 1: im via K2)
                e_tiles = [[None] * 4 for _ in range(2)]
                for comp in range(2):
                    for kc4 in range(4):
                        pss = psS.tile([128, T], FP32, tag="psS")
                        nc.tensor.matmul(
                            pss[:],
                            k_t[comp][:, kc4 * 128:(kc4 + 1) * 128],
                            q_s[:, h, :],
                            start=True, stop=True,
                        )
                        et = e_pool.tile([128, T], BF16, tag="e")
                        nc.scalar.activation(et[:], pss[:], AF.Exp)
                        e_tiles[comp][kc4] = et

                # softmax denominators -> fast reciprocal -> broadcast
                bc = []
                for comp in range(2):
                    psd = psD.tile([1, T], FP32, tag="psD")
                    for kc4 in range(4):
                        nc.tensor.matmul(
                            psd[:], ones_b[:], e_tiles[comp][kc4],
                            start=(kc4 == 0), stop=(kc4 == 3),
                        )
                    rec = rec_pool.tile([1, T], FP32, tag="rec")
                    nc.vector.reciprocal_approx_fast(rec[:], psd[:])
                    bct = bc_pool.tile([128, T], FP32, tag="bc")
                    nc.gpsimd.partition_broadcast(bct[:], rec[:])
                    bc.append(bct)

                # AV: two accumulation groups (er-part needs /dr, ei-part /di)
                pso = []
                for comp in range(2):
                    p = psO.tile([128, T], FP32, tag="psO")
                    for kc4 in range(4):
                        base = par * 256 + comp * 128
                        nc.tensor.matmul(
                            p[:],
                            v_cur[:, kc4, base:base + 128],
                            e_tiles[comp][kc4],
                            start=(kc4 == 0), stop=(kc4 == 3),
                        )
                    pso.append(p)

                # eviction: comb = pso0/d_r + pso1/d_i + obias; rows
                # [Or(0:64); Oi(64:128)]; query residual fused here.
                c = h // 2
                ta = tt_pool.tile([128, T], FP32, tag="ta")
                tb = tt_pool.tile([128, T], FP32, tag="tb")
                comb = tt_pool.tile([128, T], BF16, tag="comb")
                nc.vector.tensor_tensor(ta[:], pso[0][:], bc[0][:], OP.mult)
                nc.vector.tensor_tensor(tb[:], pso[1][:], bc[1][:], OP.mult)
                nc.vector.scalar_tensor_tensor(
                    comb[:], ta[:], ob_s[:, h:h + 1], tb[:], OP.add, OP.add
                )
                if par == 0:
                    dsl, cc = slice(0, 64), c          # direct Or
                    ssl, sc = slice(64, 128), 8 + c    # staged Oi
                    msl = slice(0, 64)
                else:
                    dsl, cc = slice(64, 128), 8 + c    # direct Oi
                    ssl, sc = slice(0, 64), c          # staged Or
                    msl = slice(64, 128)
                nc.vector.tensor_tensor(
                    o_s[dsl, cc, :], comb[dsl, :], zq_s[dsl, cc, :], OP.add
                )
                stg = st_pool.tile([128, T], BF16, tag="stg")
                nc.sync.dma_start(stg[msl, :], comb[ssl, :])
                nc.vector.tensor_tensor(
                    o_s[msl, sc, :], stg[msl, :], zq_s[msl, sc, :], OP.add
                )

        q_cm.__exit__(None, None, None)

        # =============== LayerNorm helper ===============
        def btt(out_ap, in0_ap, in1_ap, op):
            """tensor_tensor with in1 broadcast along free dims of in0."""
            a, b2 = broadcast_tensor_aps(in0_ap, in1_ap)
            nc.vector.tensor_tensor(out_ap, a, b2, op)

        def ln_sums(src_t, psum_pool, sqp):
            """Per-comp mean and square sums (PE ones-matmuls) of a
            [128, 16, T] tile."""
            ps_mean = []
            ps_sq = []
            for comp in range(2):
                pm = psum_pool.tile([1, T], FP32, tag="lnpm")
                for c8 in range(8):
                    nc.tensor.matmul(
                        pm[:], ones_b[:], src_t[:, comp * 8 + c8, :],
                        start=(c8 == 0), stop=(c8 == 7),
                    )
                ps_mean.append(pm)
                pq = psum_pool.tile([1, T], FP32, tag="lnpq")
                for g4 in range(2):
                    sq = sqp.tile([128, 4, T], BF16, tag="sq")
                    nc.scalar.activation(
                        sq[:], src_t[:, comp * 8 + g4 * 4:comp * 8 + g4 * 4 + 4, :],
                        AF.Square,
                    )
                    for j in range(4):
                        nc.tensor.matmul(
                            pq[:], ones_b[:], sq[:, j, :],
                            start=(g4 == 0 and j == 0), stop=(g4 == 1 and j == 3),
                        )
                ps_sq.append(pq)
            return ps_mean, ps_sq

        def ln_stats(ps_mean, ps_sq, small, bcast):
            """mean/sumsq -> bf16 broadcast tiles of rstd and mean*rstd."""
            bcs = []
            for comp in range(2):
                mean = small.tile([1, T], FP32, tag="mean")
                nc.scalar.activation(
                    mean[:], ps_mean[comp][:], AF.Identity, scale=1.0 / D
                )
                m2 = small.tile([1, T], FP32, tag="m2")
                nc.scalar.activation(m2[:], mean[:], AF.Square)
                var = small.tile([1, T], FP32, tag="var")
                nc.vector.scalar_tensor_tensor(
                    var[:], ps_sq[comp][:], 1.0 / D, m2[:], OP.mult, OP.subtract
                )
                sstd = small.tile([1, T], FP32, tag="sstd")
                nc.scalar.activation(sstd[:], var[:], AF.Sqrt, bias=eps_t[0:1, :])
                rstd = small.tile([1, T], FP32, tag="rstd")
                nc.vector.reciprocal_approx_fast(rstd[:], sstd[:])
                rstd_b = small.tile([1, T], BF16, tag="rstdb")
                nc.vector.tensor_copy(rstd_b[:], rstd[:])
                mr = small.tile([1, T], BF16, tag="mr")
                nc.vector.tensor_tensor(mr[:], mean[:], rstd[:], OP.mult)
                br = bcast.tile([128, 1, T], BF16, tag="br")
                nc.gpsimd.partition_broadcast(br[:, 0, :], rstd_b[:])
                bm = bcast.tile([128, 1, T], BF16, tag="bm")
                nc.gpsimd.partition_broadcast(bm[:, 0, :], mr[:])
                bcs.append((br, bm))
            return bcs

        def ln_normalize(src_t, dst_t, idx, bcs, sqp,
                         res_t=None, out_fp32=False, dma_out=None):
            out_dt = FP32 if out_fp32 else BF16
            for comp in range(2):
                br, bm = bcs[comp]
                for g4 in range(2):
                    c0 = comp * 8 + g4 * 4
                    t1 = sqp.tile([128, 4, T], BF16, tag="lnt1")
                    btt(t1[:], src_t[:, c0:c0 + 4, :], br[:], OP.mult)
                    vh = sqp.tile([128, 4, T], BF16, tag="lnvh")
                    btt(vh[:], t1[:], bm[:], OP.subtract)
                    # affine (·g + b): Act for comp 0, DVE for comp 1
                    if res_t is None and dma_out is None:
                        aff = None
                        aff_dst = lambda j: dst_t[:, c0 + j, :]
                    else:
                        aff = sqp.tile([128, 4, T], out_dt, tag="lnaf")
                        aff_dst = lambda j: aff[:, j, :]
                    for j in range(4):
                        g_ap, b_ap = ln_gb(idx, comp, g4 * 4 + j)
                        if comp == 0:
                            nc.scalar.activation(
                                aff_dst(j), vh[:, j, :], AF.Identity,
                                bias=b_ap, scale=g_ap,
                            )
                        else:
                            nc.vector.tensor_scalar(
                                aff_dst(j), vh[:, j, :], g_ap, b_ap,
                                OP.mult, OP.add,
                            )
                    if res_t is not None:
                        nc.vector.tensor_tensor(
                            dst_t[:, c0:c0 + 4, :], aff[:],
                            res_t[:, c0:c0 + 4, :], OP.add,
                        )
                    elif dma_out is not None:
                        dma_out(c0, aff)

        def layer_norm(src_t, dst_t, idx, psum_pool, small, bcast, sqp,
                       res_t=None, out_fp32=False, dma_out=None):
            ps_mean, ps_sq = ln_sums(src_t, psum_pool, sqp)
            bcs = ln_stats(ps_mean, ps_sq, small, bcast)
            ln_normalize(src_t, dst_t, idx, bcs, sqp,
                         res_t=res_t, out_fp32=out_fp32, dma_out=dma_out)

        # =============== Phase C: two layernorms ===============
        # prefetch the first c_fc weight chunks while the LNs run (DMA is
        # otherwise idle here and c_fc would cold-start on weights)
        wfc_pre = {}
        for oc in range(2):
            for g in range(3):
                wt = wfc_pool.tile([128, 8, 128], BF16, tag="wfc")
                nc.sync.dma_start(wt[:], wfc_d[g, oc])
                wfc_pre[(g, oc)] = wt

        with (
            tc.tile_pool(name="lnsq", bufs=3) as sq_pool,
            tc.tile_pool(name="lnsm", bufs=1) as small_pool,
            tc.tile_pool(name="lnbc", bufs=4) as bc2_pool,
            tc.tile_pool(name="psC", bufs=4, space="PSUM") as psC,
        ):
            # LN#0 over (attn_out + query) [already fused], + x residual,
            # written into zx_s (x2pre)
            layer_norm(
                o_s, zx_s, 0, psC, small_pool, bc2_pool, sq_pool, res_t=zx_s,
            )
            # LN#1 over x2pre -> x2n
            layer_norm(
                zx_s, x2n, 1, psC, small_pool, bc2_pool, sq_pool,
            )

        o_cm.__exit__(None, None, None)
        zq_cm.__exit__(None, None, None)
        zx_cm.__exit__(None, None, None)

        # =============== Phase D: complex MLP (Gauss 3-mult) ===============
        with (
            tc.tile_pool(name="xsump", bufs=1) as xsum_pool,
            tc.tile_pool(name="hp", bufs=1) as h_pool,
            tc.tile_pool(name="mrt", bufs=2) as mr_pool,
            tc.tile_pool(name="psF", bufs=6, space="PSUM") as psF,
        ):
            xsum = xsum_pool.tile([128, 8, T], BF16, name="xsum")
            for c8 in range(8):
                nc.vector.tensor_tensor(
                    xsum[:, c8, :], x2n[:, c8, :], x2n[:, 8 + c8, :], OP.add
                )

            hr_t = h_pool.tile([128, OC_H, T], BF16, name="hr")
            hi_t = h_pool.tile([128, OC_H, T], BF16, name="hi")
            hs_t = h_pool.tile([128, OC_H, T], BF16, name="hs")

            # c_fc: per out chunk, three Gauss matmul groups
            for oc in range(OC_H):
                pss = []
                for g in range(3):
                    if (g, oc) in wfc_pre:
                        wt = wfc_pre[(g, oc)]
                    else:
                        wt = wfc_pool.tile([128, 8, 128], BF16, tag="wfc")
                        nc.sync.dma_start(wt[:], wfc_d[g, oc])
                    p = psF.tile([128, T], FP32, tag="psF")
                    src_base = (0, 8, 0)[g]
                    src = x2n if g < 2 else xsum
                    for kc in range(8):
                        nc.tensor.matmul(
                            p[:], wt[:, kc, :],
                            (src[:, src_base + kc, :] if g < 2
                             else xsum[:, kc, :]),
                            start=(kc == 0), stop=(kc == 7),
                        )
                    pss.append(p)
                # Hr = (T1 + br) - T2 ; Hi = ((T3 + bi) - T1) - T2
                # (DVE reads at most one PSUM operand: evict T1 via Act first)
                t1sb = mr_pool.tile([128, T], FP32, tag="t1sb")
                nc.scalar.activation(t1sb[:], pss[0][:], AF.Copy)
                nc.vector.scalar_tensor_tensor(
                    hr_t[:, oc, :], t1sb[:], bfc_s[:, oc:oc + 1], pss[1][:],
                    OP.add, OP.subtract,
                )
                tmp = mr_pool.tile([128, T], FP32, tag="gtmp")
                nc.vector.scalar_tensor_tensor(
                    tmp[:], pss[2][:], bfc_s[:, OC_H + oc:OC_H + oc + 1],
                    t1sb[:], OP.add, OP.subtract,
                )
                nc.vector.tensor_tensor(
                    hi_t[:, oc, :], tmp[:], pss[1][:], OP.subtract
                )
                # modReLU: hr += |h| (0.5 folded into wpj); hs = hr' + hi
                sq1 = mr_pool.tile([128, T], FP32, tag="mr1")
                nc.scalar.activation(sq1[:], hr_t[:, oc, :], AF.Square)
                sq2 = mr_pool.tile([128, T], FP32, tag="mr2")
                nc.scalar.activation(sq2[:], hi_t[:, oc, :], AF.Square)
                nc.vector.tensor_tensor(sq1[:], sq1[:], sq2[:], OP.add)
                mag = mr_pool.tile([128, T], BF16, tag="mag")
                nc.scalar.activation(mag[:], sq1[:], AF.Sqrt)
                nc.vector.tensor_tensor(
                    hr_t[:, oc, :], hr_t[:, oc, :], mag[:], OP.add
                )
                nc.vector.tensor_tensor(
                    hs_t[:, oc, :], hr_t[:, oc, :], hi_t[:, oc, :], OP.add
                )

            # c_proj: per out chunk pc, U1/U2/U3 Gauss groups; final
            # bias + x2n residual fused into eviction -> y_pre
            for pc in range(8):
                ups = []
                for g, hsrc in ((0, hr_t), (1, hi_t), (2, hs_t)):
                    wt = wpj_pool.tile([128, KC_H, 128], BF16, tag="wpj")
                    nc.sync.dma_start(wt[:], wpj_d[g, pc])
                    p = psF.tile([128, T], FP32, tag="psF")
                    for kc in range(KC_H):
                        nc.tensor.matmul(
                            p[:], wt[:, kc, :], hsrc[:, kc, :],
                            start=(kc == 0), stop=(kc == KC_H - 1),
                        )
                    ups.append(p)
                # Mr = (U1 + bpr) - U2 (+ x2n_r)
                u1sb = mr_pool.tile([128, T], FP32, tag="u1sb")
                nc.scalar.activation(u1sb[:], ups[0][:], AF.Copy)
                tmp = mr_pool.tile([128, T], BF16, tag="gtmp2")
                nc.vector.scalar_tensor_tensor(
                    tmp[:], u1sb[:], bp_s[:, pc:pc + 1], ups[1][:],
                    OP.add, OP.subtract,
                )
                nc.vector.tensor_tensor(
                    y_pre[:, pc, :], tmp[:], x2n[:, pc, :], OP.add
                )
                # Mi = ((U3 + bpi) - U1) - U2 (+ x2n_i)
                tmp2 = mr_pool.tile([128, T], FP32, tag="gtmp3")
                nc.vector.scalar_tensor_tensor(
                    tmp2[:], ups[2][:], bp_s[:, 8 + pc:8 + pc + 1], u1sb[:],
                    OP.add, OP.subtract,
                )
                tmp3 = mr_pool.tile([128, T], BF16, tag="gtmp4")
                nc.vector.tensor_tensor(tmp3[:], tmp2[:], ups[1][:], OP.subtract)
                nc.vector.tensor_tensor(
                    y_pre[:, 8 + pc, :], tmp3[:], x2n[:, 8 + pc, :], OP.add
                )

        x2n_cm.__exit__(None, None, None)
        wpj_cm.__exit__(None, None, None)
        wfc_cm.__exit__(None, None, None)

        # =============== final layernorm + store ===============
        with (
            tc.tile_pool(name="lnsq2", bufs=2) as sq2_pool,
            tc.tile_pool(name="lnsm2", bufs=1) as small2_pool,
            tc.tile_pool(name="lnbc2", bufs=4) as bc3_pool,
            tc.tile_pool(name="psC2", bufs=4, space="PSUM") as psC2,
        ):
            layer_norm(
                y_pre, None, 2, psC2, small2_pool, bc3_pool, sq2_pool,
                out_fp32=True,
                dma_out=lambda c0, aff: nc.sync.dma_start(
                    y_d[:, c0:c0 + 4, :], aff[:]
                ),
            )

        yp_cm.__exit__(None, None, None)
        consts_cm.__exit__(None, None, None)

    nc.compile()
    if not nc.is_finalized():
        nc.finalize()
    return nc


def _stackT(w):
    """[F, Din, 2] torch-layout complex weight -> [2*Din, 2*F] stacked lhsT."""
    wr = w[..., 0].astype(np.float32)
    wi = w[..., 1].astype(np.float32)
    top = np.concatenate([wr.T, wi.T], axis=1)
    bot = np.concatenate([-wi.T, wr.T], axis=1)
    return np.concatenate([top, bot], axis=0)


def _prep_weights(wq, bq, wk, bk, wv, bv, w_fc, b_fc, w_proj, b_proj, ln_g, ln_b):
    qcols = np.concatenate(
        [np.concatenate([np.arange(h * 64, h * 64 + 64),
                         1024 + np.arange(h * 64, h * 64 + 64)]) for h in range(NH)]
    )
    scale = np.float32(1.0 / np.sqrt(DH))

    sq = _stackT(wq) * scale
    wq_t = np.ascontiguousarray(
        sq[:, qcols].reshape(KC_D, 128, MC_D, 128).transpose(2, 1, 0, 3)
    ).astype(NPBF)
    bq_l = (np.concatenate([bq[:, 0], bq[:, 1]]) * scale)[qcols]
    bq_a = np.ascontiguousarray(
        bq_l.reshape(MC_D, 128).T.astype(np.float32)
    )

    sk = _stackT(wk)
    bkst = np.concatenate([bk[:, 0], bk[:, 1]]).astype(np.float32)
    wk_full = sk[:, qcols].copy()           # [2048, 2048]: per head [Kr | Ki]
    bk_l = bkst[qcols].copy()
    for h in range(NH):
        wk_full[:, h * 128 + 64:h * 128 + 128] *= -1.0   # -> [Kr | -Ki]
        bk_l[h * 128 + 64:h * 128 + 128] *= -1.0
    wk_t = np.ascontiguousarray(
        wk_full.reshape(KC_D, 128, MC_D, 128).transpose(2, 1, 0, 3)
    ).astype(NPBF)
    bk_a = np.ascontiguousarray(bk_l.reshape(MC_D, 128).T.astype(np.float32))

    sv = _stackT(wv)
    svq = sv[:, qcols]                       # [2048, 2048]
    wv_t = np.ascontiguousarray(
        svq.reshape(KC_D, 128, NH // 2, 256).transpose(2, 1, 0, 3)
    ).astype(NPBF)
    # obias: V bias deferred through softmax; per head column:
    # rows 0:64 = bvr - bvi (Or), rows 64:128 = bvr + bvi (Oi)
    ob = np.empty((128, NH), dtype=np.float32)
    bvr, bvi = bv[:, 0].astype(np.float32), bv[:, 1].astype(np.float32)
    for h in range(NH):
        sl = slice(h * 64, h * 64 + 64)
        ob[0:64, h] = bvr[sl] - bvi[sl]
        ob[64:128, h] = bvr[sl] + bvi[sl]

    # Gauss c_fc: blocks Wr^T, Wi^T, (Wr+Wi)^T  [1024, 4096]
    fr = w_fc[..., 0].astype(np.float32).T
    fi = w_fc[..., 1].astype(np.float32).T
    wfc_t = np.ascontiguousarray(
        np.stack([fr, fi, fr + fi])
        .reshape(3, 8, 128, OC_H, 128).transpose(0, 3, 2, 1, 4)
    ).astype(NPBF)
    bfc_a = np.ascontiguousarray(
        np.concatenate([b_fc[:, 0], b_fc[:, 1]])
        .reshape(2 * OC_H, 128).T.astype(np.float32)
    )

    # Gauss c_proj (0.5 of modReLU folded into weights): [4096, 1024] blocks
    pr = (w_proj[..., 0].astype(np.float32) * 0.5).T
    pi = (w_proj[..., 1].astype(np.float32) * 0.5).T
    wpj_t = np.ascontiguousarray(
        np.stack([pr, pi, pr + pi])
        .reshape(3, KC_H, 128, 8, 128).transpose(0, 3, 2, 1, 4)
    ).astype(NPBF)
    bp_a = np.ascontiguousarray(
        np.concatenate([b_proj[:, 0], b_proj[:, 1]])
        .reshape(16, 128).T.astype(np.float32)
    )

    lng_a = np.ascontiguousarray(
        ln_g.astype(np.float32).reshape(3, 2, 8, 128).transpose(3, 0, 1, 2).reshape(128, 48)
    )
    lnb_a = np.ascontiguousarray(
        ln_b.astype(np.float32).reshape(3, 2, 8, 128).transpose(3, 0, 1, 2).reshape(128, 48)
    )
    return {
        "wq": wq_t, "bq": bq_a, "wk": wk_t, "bk": bk_a, "wv": wv_t, "ob": ob,
        "wfc": wfc_t, "bfc": bfc_a, "wpj": wpj_t, "bp": bp_a,
        "lng": lng_a, "lnb": lnb_a,
    }


_NC_CACHE = {}


def kernel(**inputs):
    if "nc" not in _NC_CACHE:
        _NC_CACHE["nc"] = _build_nc()
    nc = _NC_CACHE["nc"]

    x = np.asarray(inputs["x"], dtype=np.float32)
    query = np.asarray(inputs["query"], dtype=np.float32)
    shared = _prep_weights(
        np.asarray(inputs["wq"]), np.asarray(inputs["bq"]),
        np.asarray(inputs["wk"]), np.asarray(inputs["bk"]),
        np.asarray(inputs["wv"]), np.asarray(inputs["bv"]),
        np.asarray(inputs["w_fc"]), np.asarray(inputs["b_fc"]),
        np.asarray(inputs["w_proj"]), np.asarray(inputs["b_proj"]),
        np.asarray(inputs["ln_g"]), np.asarray(inputs["ln_b"]),
    )

    in_maps = []
    for b in range(B):
        zq = np.ascontiguousarray(
            np.concatenate([query[b, :, :, 0].T, query[b, :, :, 1].T], axis=0)
            .reshape(KC_D, 128, T).transpose(1, 0, 2)
        ).astype(NPBF)
        zx = np.ascontiguousarray(
            np.concatenate([x[b, :, :, 0].T, x[b, :, :, 1].T], axis=0)
            .reshape(KC_D, 128, T).transpose(1, 0, 2)
        ).astype(NPBF)
        m = {"zq": zq, "zx": zx}
        m.update(shared)
        in_maps.append(m)

    import os
    trace = bool(os.environ.get("KERNEL_TRACE"))
    res = run_bass_kernel_spmd(nc, in_maps, list(range(N_CORES)), trace=trace)
    _NC_CACHE["exec_time_ns"] = res.exec_time_ns
    out = np.empty((B, S, D, 2), dtype=np.float32)
    for b in range(B):
        yb = res.results[b]["y"].transpose(1, 0, 2).reshape(D2, T)
        out[b, :, :, 0] = yb[:D, :].T
        out[b, :, :, 1] = yb[D:, :].T
    return out


if __name__ == "__main__":
    rng = np.random.default_rng(0)
    f = np.float32
    demo = {
        "x": rng.standard_normal((B, S, D, 2), dtype=f),
        "query": rng.standard_normal((B, S, D, 2), dtype=f),
        "wq": rng.standard_normal((D, D, 2), dtype=f) * 0.02,
        "bq": rng.standard_normal((D, 2), dtype=f) * 0.02,
        "wk": rng.standard_normal((D, D, 2), dtype=f) * 0.02,
        "bk": rng.standard_normal((D, 2), dtype=f) * 0.02,
        "wv": rng.standard_normal((D, D, 2), dtype=f) * 0.02,
        "bv": rng.standard_normal((D, 2), dtype=f) * 0.02,
        "w_fc": rng.standard_normal((HID, D, 2), dtype=f) * 0.02,
        "b_fc": rng.standard_normal((HID, 2), dtype=f) * 0.02,
        "w_proj": rng.standard_normal((D, HID, 2), dtype=f) * 0.02,
        "b_proj": rng.standard_normal((D, 2), dtype=f) * 0.02,
        "ln_g": np.ones((3, 2, D), dtype=f),
        "ln_b": np.zeros((3, 2, D), dtype=f),
    }
    out = kernel(**demo)
    print("out shape", out.shape)


# revision 40
# speedup vs baseline: 1.1761x; 1.0499x over previous
"""Trainium2 Bass kernel for nn_ComplexCrossAttention.

Strategy (v2):
- Data-parallel over batch B=8 across 8 NeuronCores (one batch element each,
  no collectives).
- All matmul operands are bf16 (PSUM accumulation fp32): enables FWL so
  LDWEIGHTS overlaps matmuls, and halves weight DMA vs fp32.
- QKV projections stay in the stacked-real form Z=[re;im] with prestacked
  weights; the complex MLP uses the Gauss 3-multiplication trick
  (T1=Ar Wr, T2=Ai Wi, T3=(Ar+Ai)(Wr+Wi)) cutting c_fc/c_proj PE time 25%.
- Attention per head: transposed scores St[k,q], exp straight out of PSUM,
  key-axis softmax sums via ones-matmuls, 1/denom via reciprocal_approx_fast,
  normalization + V-bias + query-residual folded into the AV eviction.
  The V bias is deferred through softmax (attention rows sum to 1):
  obias_r = bvr - bvi, obias_i = bvr + bvi added at eviction.
- Activations/weights shipped in partition-major contiguous layouts so all
  big DMAs are linear.
"""

import sys

for _p in ("/opt/trn_rl_repo",):
    if _p not in sys.path:
        sys.path.insert(0, _p)

import numpy as np
import ml_dtypes

import concourse.bass as bass
import concourse.mybir as mybir
import concourse.tile as tile
from concourse import bacc
from concourse.bass import broadcast_tensor_aps
from concourse.bass_utils import run_bass_kernel_spmd

BF16 = mybir.dt.bfloat16
FP32 = mybir.dt.float32
AF = mybir.ActivationFunctionType
OP = mybir.AluOpType

B, S, D = 8, 512, 1024
NH, DH = 16, 64
HID = 4096
T = S
N_CORES = 8
D2 = 2 * D       # 2048 stacked features
KC_D = D2 // 128   # 16 contraction chunks of the model dim
MC_D = D2 // 128   # 16 chunks of the model dim
OC_H = HID // 128  # 32 out chunks of one MLP hidden component
KC_H = HID // 128  # 32 contraction chunks of one hidden component
EPS = 1e-5
NPBF = ml_dtypes.bfloat16


def _build_nc():
    nc = bacc.Bacc(None, target_bir_lowering=False, debug=False)

    zq_d = nc.dram_tensor("zq", [128, KC_D, T], BF16, kind="ExternalInput")
    zx_d = nc.dram_tensor("zx", [128, KC_D, T], BF16, kind="ExternalInput")
    wq_d = nc.dram_tensor("wq", [MC_D, 128, KC_D, 128], BF16, kind="ExternalInput")
    wk_d = nc.dram_tensor("wk", [MC_D, 128, KC_D, 128], BF16, kind="ExternalInput")
    wv_d = nc.dram_tensor("wv", [NH // 2, 128, KC_D, 256], BF16, kind="ExternalInput")
    wfc_d = nc.dram_tensor("wfc", [3, OC_H, 128, 8, 128], BF16, kind="ExternalInput")
    wcb_d = nc.dram_tensor("wcb", [MC_D, 128, KC_D, 128], BF16, kind="ExternalInput")
    wmg_d = nc.dram_tensor("wmg", [MC_D, 128, KC_H, 128], BF16, kind="ExternalInput")
    bq_d = nc.dram_tensor("bq", [128, MC_D], FP32, kind="ExternalInput")
    bk_d = nc.dram_tensor("bk", [128, MC_D], FP32, kind="ExternalInput")
    ob_d = nc.dram_tensor("ob", [128, NH], FP32, kind="ExternalInput")
    bfc_d = nc.dram_tensor("bfc", [128, 2 * OC_H], FP32, kind="ExternalInput")
    cb_d = nc.dram_tensor("cb", [128, MC_D], FP32, kind="ExternalInput")
    lng_d = nc.dram_tensor("lng", [128, 48], FP32, kind="ExternalInput")
    lnb_d = nc.dram_tensor("lnb", [128, 48], FP32, kind="ExternalInput")
    y_d = nc.dram_tensor("y", [128, MC_D, T], FP32, kind="ExternalOutput")

    with tile.TileContext(nc) as tc:
        consts_cm = tc.tile_pool(name="consts", bufs=1)
        consts = consts_cm.__enter__()

        # full-width ones: NumWeights=128 keeps FWL on for the sum-matmuls,
        # and every PSUM row of the result holds the full partition-sum,
        # which makes the separate gpsimd broadcasts unnecessary.
        ones_b = consts.tile([128, 128], BF16)
        nc.vector.memset(ones_b[:], 1.0)
        eps_t = consts.tile([128, 1], FP32)
        nc.vector.memset(eps_t[:], EPS)
        bq_s = consts.tile([128, MC_D], FP32)
        nc.sync.dma_start(bq_s[:], bq_d[:])
        bk_s = consts.tile([128, MC_D], FP32)
        nc.sync.dma_start(bk_s[:], bk_d[:])
        ob_s = consts.tile([128, NH], FP32)
        nc.sync.dma_start(ob_s[:], ob_d[:])
        bfc_s = consts.tile([128, 2 * OC_H], FP32)
        nc.sync.dma_start(bfc_s[:], bfc_d[:])
        cb_s = consts.tile([128, MC_D], FP32)
        nc.sync.dma_start(cb_s[:], cb_d[:])
        lng_s = consts.tile([128, 48], FP32)
        nc.sync.dma_start(lng_s[:], lng_d[:])
        lnb_s = consts.tile([128, 48], FP32)
        nc.sync.dma_start(lnb_s[:], lnb_d[:])

        def ln_gb(idx, comp, c8):
            j = idx * 16 + comp * 8 + c8
            return lng_s[:, j:j + 1], lnb_s[:, j:j + 1]

        # ---- long-lived activation pools (manually scoped, LIFO order:
        # entered in reverse order of release) ----
        yp_cm = tc.tile_pool(name="yp", bufs=1)
        yp_pool = yp_cm.__enter__()
        y_pre = yp_pool.tile([128, MC_D, T], BF16, name="y_pre")

        wfc_cm = tc.tile_pool(name="wfcp", bufs=6)
        wfc_pool = wfc_cm.__enter__()
        wpj_cm = tc.tile_pool(name="wpjp", bufs=2)
        wpj_pool = wpj_cm.__enter__()

        x2n_cm = tc.tile_pool(name="x2n", bufs=1)
        x2n_pool = x2n_cm.__enter__()
        x2n = x2n_pool.tile([128, MC_D, T], BF16, name="x2n")

        zx_cm = tc.tile_pool(name="zx", bufs=1)
        zx_pool = zx_cm.__enter__()
        zx_s = zx_pool.tile([128, KC_D, T], BF16, name="zx_s")

        zq_cm = tc.tile_pool(name="zq", bufs=1)
        zq_pool = zq_cm.__enter__()
        zq_s = zq_pool.tile([128, KC_D, T], BF16, name="zq_s")
        for i in range(4):
            nc.sync.dma_start(
                zq_s[:, i * 4:(i + 1) * 4, :], zq_d[:, i * 4:(i + 1) * 4, :]
            )

        o_cm = tc.tile_pool(name="op", bufs=1)
        o_pool = o_cm.__enter__()
        o_s = o_pool.tile([128, MC_D, T], BF16, name="o_s")

        q_cm = tc.tile_pool(name="qp", bufs=1)
        q_pool = q_cm.__enter__()
        q_s = q_pool.tile([128, NH, T], BF16, name="q_s")

        # =============== Phase A: Q projection (feature-major) ===============
        with (
            tc.tile_pool(name="wqp", bufs=3) as wq_pool,
            tc.tile_pool(name="psA", bufs=4, space="PSUM") as psA,
        ):
            for mc in range(MC_D):
                wt = wq_pool.tile([128, KC_D, 128], BF16, tag="wq")
                nc.sync.dma_start(wt[:], wq_d[mc])
                if mc == 1:
                    # x load deferred behind the first Q-proj weights so the
                    # PE isn't starved at kernel start (x isn't needed until
                    # the K/V projections in phase B).
                    for i in range(4):
                        nc.sync.dma_start(
                            zx_s[:, i * 4:(i + 1) * 4, :],
                            zx_d[:, i * 4:(i + 1) * 4, :],
                        )
                ps = psA.tile([128, T], FP32, tag="psA")
                for kc in range(KC_D):
                    nc.tensor.matmul(
                        ps[:], wt[:, kc, :], zq_s[:, kc, :],
                        start=(kc == 0), stop=(kc == KC_D - 1),
                    )
                nc.scalar.activation(
                    q_s[:, mc, :], ps[:], AF.Identity, bias=bq_s[:, mc:mc + 1]
                )

        # =============== Phase B: attention, head-streamed ===============
        with (
            tc.tile_pool(name="wkp", bufs=2) as wk_pool,
            tc.tile_pool(name="wvp", bufs=2) as wv_pool,
            tc.tile_pool(name="kp", bufs=4) as k_pool,
            tc.tile_pool(name="vp", bufs=2) as v_pool,
            tc.tile_pool(name="ep", bufs=10) as e_pool,
            tc.tile_pool(name="ttp", bufs=4) as tt_pool,
            tc.tile_pool(name="stp", bufs=2) as st_pool,
            tc.tile_pool(name="bcp", bufs=3) as bc_pool,
            tc.tile_pool(name="psK", bufs=1, space="PSUM") as psK,
            tc.tile_pool(name="psV", bufs=1, space="PSUM") as psV,
            tc.tile_pool(name="psS", bufs=2, space="PSUM") as psS,
            tc.tile_pool(name="psO", bufs=2, space="PSUM") as psO,
            tc.tile_pool(name="psD", bufs=2, space="PSUM") as psD,
        ):
            v_cur = None
            for h in range(NH):
                hp, par = divmod(h, 2)
                if par == 0:
                    # V projection for the head pair (token-major), no bias
                    # (deferred through softmax into obias at eviction).
                    wvt = wv_pool.tile([128, KC_D, 256], BF16, tag="wv")
                    nc.sync.dma_start(wvt[:], wv_d[hp])
                    v_cur = v_pool.tile([128, 4, 512], BF16, tag="v")
                    for tcb in range(4):
                        psv = psV.tile([128, 256], FP32, tag="psV")
                        for kc in range(KC_D):
                            nc.tensor.matmul(
                                psv[:],
                                zx_s[:, kc, tcb * 128:(tcb + 1) * 128],
                                wvt[:, kc, :],
                                start=(kc == 0), stop=(kc == KC_D - 1),
                            )
                        for sub in range(2):
                            base = sub * 256
                            # V1 = [Vr | Vi]
                            nc.vector.tensor_copy(
                                v_cur[:, tcb, base:base + 128],
                                psv[:, sub * 128:(sub + 1) * 128],
                            )
                            # V2 = [-Vi | Vr]
                            nc.scalar.activation(
                                v_cur[:, tcb, base + 128:base + 192],
                                psv[:, sub * 128 + 64:sub * 128 + 128],
                                AF.Identity, scale=-1.0,
                            )
                            nc.scalar.activation(
                                v_cur[:, tcb, base + 192:base + 256],
                                psv[:, sub * 128:sub * 128 + 64],
                                AF.Copy,
                            )

                # K1 = [Kr; -Ki] projection (feature-major); K2 = [Ki; Kr]
                wkt = wk_pool.tile([128, KC_D, 128], BF16, tag="wk")
                nc.sync.dma_start(wkt[:], wk_d[h])
                k1 = k_pool.tile([128, T], BF16, tag="k")
                ps = psK.tile([128, T], FP32, tag="psK")
                for kc in range(KC_D):
                    nc.tensor.matmul(
                        ps[:], wkt[:, kc, :], zx_s[:, kc, :],
                        start=(kc == 0), stop=(kc == KC_D - 1),
                    )
                nc.scalar.activation(
                    k1[:], ps[:], AF.Identity, bias=bk_s[:, h:h + 1]
                )
                k2 = k_pool.tile([128, T], BF16, tag="k")
                nc.sync.dma_start(k2[0:64, :], k1[64:128, :])
                nc.vector.tensor_scalar_mul(k2[0:64, :], k2[0:64, :], -1.0)
                nc.sync.dma_start(k2[64:128, :], k1[0:64, :])
                k_t = [k1, k2]

                # transposed scores + exp (comp 0: re via K1, comp 1: im via K2)
                e_tiles = [[None] * 4 for _ in range(2)]
                for comp in range(2):
                    for kc4 in range(4):
                        pss = psS.tile([128, T], FP32, tag="psS")
                        nc.tensor.matmul(
                            pss[:],
                            k_t[comp][:, kc4 * 128:(kc4 + 1) * 128],
                            q_s[:, h, :],
                            start=True, stop=True,
                        )
                        et = e_pool.tile([128, T], BF16, tag="e")
                        nc.scalar.activation(et[:], pss[:], AF.Exp)
                        e_tiles[comp][kc4] = et

                # softmax denominators (every PSUM row = the full key-sum)
                # -> fast reciprocal, already broadcast across partitions
                bc = []
                for comp in range(2):
                    psd = psD.tile([128, T], FP32, tag="psD")
                    for kc4 in range(4):
                        nc.tensor.matmul(
                            psd[:], ones_b[:], e_tiles[comp][kc4],
                            start=(kc4 == 0), stop=(kc4 == 3),
                        )
                    bct = bc_pool.tile([128, T], FP32, tag="bc")
                    nc.vector.reciprocal_approx_fast(bct[:], psd[:])
                    bc.append(bct)

                # AV: two accumulation groups (er-part needs /dr, ei-part /di)
                pso = []
                for comp in range(2):
                    p = psO.tile([128, T], FP32, tag="psO")
                    for kc4 in range(4):
                        base = par * 256 + comp * 128
                        nc.tensor.matmul(
                            p[:],
                            v_cur[:, kc4, base:base + 128],
                            e_tiles[comp][kc4],
                            start=(kc4 == 0), stop=(kc4 == 3),
                        )
                    pso.append(p)

                # eviction: comb = pso0/d_r + pso1/d_i + obias; rows
                # [Or(0:64); Oi(64:128)]; query residual fused here.
                c = h // 2
                ta = tt_pool.tile([128, T], FP32, tag="ta")
                tb = tt_pool.tile([128, T], FP32, tag="tb")
                comb = tt_pool.tile([128, T], BF16, tag="comb")
                nc.vector.tensor_tensor(ta[:], pso[0][:], bc[0][:], OP.mult)
                nc.vector.tensor_tensor(tb[:], pso[1][:], bc[1][:], OP.mult)
                nc.vector.scalar_tensor_tensor(
                    comb[:], ta[:], ob_s[:, h:h + 1], tb[:], OP.add, OP.add
                )
                if par == 0:
                    dsl, cc = slice(0, 64), c          # direct Or
                    ssl, sc = slice(64, 128), 8 + c    # staged Oi
                    msl = slice(0, 64)
                else:
                    dsl, cc = slice(64, 128), 8 + c    # direct Oi
                    ssl, sc = slice(0, 64), c          # staged Or
                    msl = slice(64, 128)
                nc.vector.tensor_tensor(
                    o_s[dsl, cc, :], comb[dsl, :], zq_s[dsl, cc, :], OP.add
                )
                stg = st_pool.tile([128, T], BF16, tag="stg")
                nc.sync.dma_start(stg[msl, :], comb[ssl, :])
                nc.vector.tensor_tensor(
                    o_s[msl, sc, :], stg[msl, :], zq_s[msl, sc, :], OP.add
                )

        q_cm.__exit__(None, None, None)

        # =============== LayerNorm helper ===============
        def btt(out_ap, in0_ap, in1_ap, op):
            """tensor_tensor with in1 broadcast along free dims of in0."""
            a, b2 = broadcast_tensor_aps(in0_ap, in1_ap)
            nc.vector.tensor_tensor(out_ap, a, b2, op)

        def ln_sums(src_t, psum_pool, sqp):
            """Per-comp mean and square sums (PE ones-matmuls) of a
            [128, 16, T] tile."""
            ps_mean = []
            ps_sq = []
            for comp in range(2):
                pm = psum_pool.tile([128, T], FP32, tag="lnpm")
                for c8 in range(8):
                    nc.tensor.matmul(
                        pm[:], ones_b[:], src_t[:, comp * 8 + c8, :],
                        start=(c8 == 0), stop=(c8 == 7),
                    )
                ps_mean.append(pm)
                pq = psum_pool.tile([128, T], FP32, tag="lnpq")
                for g4 in range(2):
                    sq = sqp.tile([128, 4, T], BF16, tag="sq")
                    nc.scalar.activation(
                        sq[:], src_t[:, comp * 8 + g4 * 4:comp * 8 + g4 * 4 + 4, :],
                        AF.Square,
                    )
                    for j in range(4):
                        nc.tensor.matmul(
                            pq[:], ones_b[:], sq[:, j, :],
                            start=(g4 == 0 and j == 0), stop=(g4 == 1 and j == 3),
                        )
                ps_sq.append(pq)
            return ps_mean, ps_sq

        def ln_stats(ps_mean, ps_sq, small, bcast):
            """mean/sumsq (full-width, every row identical) -> bf16 tiles of
            rstd and mean*rstd, already broadcast across partitions."""
            bcs = []
            for comp in range(2):
                mean = small.tile([128, T], FP32, tag="mean")
                nc.scalar.activation(
                    mean[:], ps_mean[comp][:], AF.Identity, scale=1.0 / D
                )
                m2 = small.tile([128, T], FP32, tag="m2")
                nc.scalar.activation(m2[:], mean[:], AF.Square)
                var = small.tile([128, T], FP32, tag="var")
                nc.vector.scalar_tensor_tensor(
                    var[:], ps_sq[comp][:], 1.0 / D, m2[:], OP.mult, OP.subtract
                )
                sstd = small.tile([128, T], FP32, tag="sstd")
                nc.scalar.activation(sstd[:], var[:], AF.Sqrt, bias=eps_t[:, :])
                rstd = small.tile([128, T], FP32, tag="rstd")
                nc.vector.reciprocal_approx_fast(rstd[:], sstd[:])
                br = bcast.tile([128, 1, T], BF16, tag="br")
                nc.vector.tensor_copy(br[:, 0, :], rstd[:])
                bm = bcast.tile([128, 1, T], BF16, tag="bm")
                nc.vector.tensor_tensor(bm[:, 0, :], mean[:], rstd[:], OP.mult)
                bcs.append((br, bm))
            return bcs

        def ln_normalize(src_t, dst_t, idx, bcs, sqp,
                         res_t=None, out_fp32=False, dma_out=None):
            out_dt = FP32 if out_fp32 else BF16
            for comp in range(2):
                br, bm = bcs[comp]
                for g4 in range(2):
                    c0 = comp * 8 + g4 * 4
                    t1 = sqp.tile([128, 4, T], BF16, tag="lnt1")
                    btt(t1[:], src_t[:, c0:c0 + 4, :], br[:], OP.mult)
                    vh = sqp.tile([128, 4, T], BF16, tag="lnvh")
                    btt(vh[:], t1[:], bm[:], OP.subtract)
                    # affine (·g + b): Act for comp 0, DVE for comp 1
                    if res_t is None and dma_out is None:
                        aff = None
                        aff_dst = lambda j: dst_t[:, c0 + j, :]
                    else:
                        aff = sqp.tile([128, 4, T], out_dt, tag="lnaf")
                        aff_dst = lambda j: aff[:, j, :]
                    for j in range(4):
                        g_ap, b_ap = ln_gb(idx, comp, g4 * 4 + j)
                        if comp == 0:
                            nc.scalar.activation(
                                aff_dst(j), vh[:, j, :], AF.Identity,
                                bias=b_ap, scale=g_ap,
                            )
                        else:
                            nc.vector.tensor_scalar(
                                aff_dst(j), vh[:, j, :], g_ap, b_ap,
                                OP.mult, OP.add,
                            )
                    if res_t is not None:
                        nc.vector.tensor_tensor(
                            dst_t[:, c0:c0 + 4, :], aff[:],
                            res_t[:, c0:c0 + 4, :], OP.add,
                        )
                    elif dma_out is not None:
                        dma_out(c0, aff)

        def layer_norm(src_t, dst_t, idx, psum_pool, small, bcast, sqp,
                       res_t=None, out_fp32=False, dma_out=None):
            ps_mean, ps_sq = ln_sums(src_t, psum_pool, sqp)
            bcs = ln_stats(ps_mean, ps_sq, small, bcast)
            ln_normalize(src_t, dst_t, idx, bcs, sqp,
                         res_t=res_t, out_fp32=out_fp32, dma_out=dma_out)

        # =============== Phase C: two layernorms ===============
        # prefetch the first c_fc weight chunks while the LNs run (DMA is
        # otherwise idle here and c_fc would cold-start on weights)
        wfc_pre = {}
        for oc in range(2):
            for g in range(3):
                wt = wfc_pool.tile([128, 8, 128], BF16, tag="wfc")
                nc.sync.dma_start(wt[:], wfc_d[g, oc])
                wfc_pre[(g, oc)] = wt

        with (
            tc.tile_pool(name="lnsq", bufs=3) as sq_pool,
            tc.tile_pool(name="lnsm", bufs=1) as small_pool,
            tc.tile_pool(name="lnbc", bufs=4) as bc2_pool,
            tc.tile_pool(name="psC", bufs=2, space="PSUM") as psC,
        ):
            # LN#0 over (attn_out + query) [already fused], + x residual,
            # written into zx_s (x2pre)
            layer_norm(
                o_s, zx_s, 0, psC, small_pool, bc2_pool, sq_pool, res_t=zx_s,
            )
            # LN#1 over x2pre -> x2n
            layer_norm(
                zx_s, x2n, 1, psC, small_pool, bc2_pool, sq_pool,
            )

        o_cm.__exit__(None, None, None)
        zq_cm.__exit__(None, None, None)
        zx_cm.__exit__(None, None, None)

        # =============== Phase D: complex MLP ===============
        # c_fc is the Gauss 3-mult form. The c_proj+modReLU linear path is
        # folded on the host: M = (0.5 Wp Wfc) x2n + 0.5 Wp |h| + cbias, so
        # only |h| is taken from the c_fc output; Wcomb (stacked complex) and
        # the |h| matmul accumulate into a single PSUM group per out chunk.
        with (
            tc.tile_pool(name="xsump", bufs=1) as xsum_pool,
            tc.tile_pool(name="hp", bufs=3) as h_pool,
            tc.tile_pool(name="magp", bufs=1) as mag_pool,
            tc.tile_pool(name="mrt", bufs=2) as mr_pool,
            tc.tile_pool(name="sqyp", bufs=2) as sqy_pool,
            tc.tile_pool(name="lnsq2", bufs=2) as sq2_pool,
            tc.tile_pool(name="lnsm2", bufs=1) as small2_pool,
            tc.tile_pool(name="lnbc2", bufs=4) as bc3_pool,
            tc.tile_pool(name="psF", bufs=4, space="PSUM") as psF,
            tc.tile_pool(name="psC2", bufs=2, space="PSUM") as psC2,
        ):
            xsum = xsum_pool.tile([128, 8, T], BF16, name="xsum")
            for c8 in range(8):
                nc.vector.tensor_tensor(
                    xsum[:, c8, :], x2n[:, c8, :], x2n[:, 8 + c8, :], OP.add
                )

            mag_t = mag_pool.tile([128, KC_H, T], BF16, name="mag")

            # c_fc: per out chunk, three Gauss matmul groups -> |h| only
            for oc in range(OC_H):
                pss = []
                for g in range(3):
                    if (g, oc) in wfc_pre:
                        wt = wfc_pre[(g, oc)]
                    else:
                        wt = wfc_pool.tile([128, 8, 128], BF16, tag="wfc")
                        nc.sync.dma_start(wt[:], wfc_d[g, oc])
                    p = psF.tile([128, T], FP32, tag="psF")
                    src_base = (0, 8, 0)[g]
                    src = x2n if g < 2 else xsum
                    for kc in range(8):
                        nc.tensor.matmul(
                            p[:], wt[:, kc, :],
                            (src[:, src_base + kc, :] if g < 2
                             else xsum[:, kc, :]),
                            start=(kc == 0), stop=(kc == 7),
                        )
                    pss.append(p)
                # Hr = (T1 + br) - T2 ; Hi = ((T3 + bi) - T1) - T2
                # (DVE reads at most one PSUM operand: evict T1 via Act first)
                t1sb = mr_pool.tile([128, T], FP32, tag="t1sb")
                nc.scalar.activation(t1sb[:], pss[0][:], AF.Copy)
                hr = h_pool.tile([128, T], BF16, tag="hr")
                nc.vector.scalar_tensor_tensor(
                    hr[:], t1sb[:], bfc_s[:, oc:oc + 1], pss[1][:],
                    OP.add, OP.subtract,
                )
                tmp = mr_pool.tile([128, T], FP32, tag="gtmp")
                nc.vector.scalar_tensor_tensor(
                    tmp[:], pss[2][:], bfc_s[:, OC_H + oc:OC_H + oc + 1],
                    t1sb[:], OP.add, OP.subtract,
                )
                hi = h_pool.tile([128, T], BF16, tag="hi")
                nc.vector.tensor_tensor(hi[:], tmp[:], pss[1][:], OP.subtract)
                # |h| = sqrt(hr^2 + hi^2)
                sq1 = mr_pool.tile([128, T], FP32, tag="mr1")
                nc.scalar.activation(sq1[:], hr[:], AF.Square)
                sq2 = mr_pool.tile([128, T], FP32, tag="mr2")
                nc.scalar.activation(sq2[:], hi[:], AF.Square)
                nc.vector.tensor_tensor(sq1[:], sq1[:], sq2[:], OP.add)
                nc.scalar.activation(mag_t[:, oc, :], sq1[:], AF.Sqrt)

            # combined projection: per stacked out chunk mc, one PSUM group
            # accumulates Wcomb·x2n (16 mm) + Wmag·|h| (32 mm); eviction adds
            # cbias + x2n residual in one DVE op. LN#2 mean sums interleave.
            psm2 = [
                psC2.tile([128, T], FP32, tag="lnpm", name=f"psm2_{i}")
                for i in range(2)
            ]
            psq2 = [
                psC2.tile([128, T], FP32, tag="lnpq", name=f"psq2_{i}")
                for i in range(2)
            ]
            for mc in range(MC_D):
                wcb = wpj_pool.tile([128, KC_D, 128], BF16, tag="wcb")
                nc.sync.dma_start(wcb[:], wcb_d[mc])
                wmg = wpj_pool.tile([128, KC_H, 128], BF16, tag="wmg")
                nc.sync.dma_start(wmg[:], wmg_d[mc])
                p = psF.tile([128, T], FP32, tag="psF")
                for kc in range(KC_D):
                    nc.tensor.matmul(
                        p[:], wcb[:, kc, :], x2n[:, kc, :],
                        start=(kc == 0), stop=False,
                    )
                for kc in range(KC_H):
                    nc.tensor.matmul(
                        p[:], wmg[:, kc, :], mag_t[:, kc, :],
                        start=False, stop=(kc == KC_H - 1),
                    )
                nc.vector.scalar_tensor_tensor(
                    y_pre[:, mc, :], p[:], cb_s[:, mc:mc + 1], x2n[:, mc, :],
                    OP.add, OP.add,
                )
                # LN#2 sums for the finished chunk
                comp, c8 = divmod(mc, 8)
                nc.tensor.matmul(
                    psm2[comp][:], ones_b[:], y_pre[:, mc, :],
                    start=(c8 == 0), stop=(c8 == 7),
                )
                sqt = sqy_pool.tile([128, T], BF16, tag="sqy")
                nc.scalar.activation(sqt[:], y_pre[:, mc, :], AF.Square)
                nc.tensor.matmul(
                    psq2[comp][:], ones_b[:], sqt[:],
                    start=(c8 == 0), stop=(c8 == 7),
                )

            # =============== final layernorm + store ===============
            bcs2 = ln_stats(psm2, psq2, small2_pool, bc3_pool)
            ln_normalize(
                y_pre, None, 2, bcs2, sq2_pool, out_fp32=True,
                dma_out=lambda c0, aff: nc.sync.dma_start(
                    y_d[:, c0:c0 + 4, :], aff[:]
                ),
            )

        x2n_cm.__exit__(None, None, None)
        wpj_cm.__exit__(None, None, None)
        wfc_cm.__exit__(None, None, None)
        yp_cm.__exit__(None, None, None)
        consts_cm.__exit__(None, None, None)

    nc.compile()
    if not nc.is_finalized():
        nc.finalize()
    return nc


def _stackT(w):
    """[F, Din, 2] torch-layout complex weight -> [2*Din, 2*F] stacked lhsT."""
    wr = w[..., 0].astype(np.float32)
    wi = w[..., 1].astype(np.float32)
    top = np.concatenate([wr.T, wi.T], axis=1)
    bot = np.concatenate([-wi.T, wr.T], axis=1)
    return np.concatenate([top, bot], axis=0)


def _prep_weights(wq, bq, wk, bk, wv, bv, w_fc, b_fc, w_proj, b_proj, ln_g, ln_b):
    qcols = np.concatenate(
        [np.concatenate([np.arange(h * 64, h * 64 + 64),
                         1024 + np.arange(h * 64, h * 64 + 64)]) for h in range(NH)]
    )
    scale = np.float32(1.0 / np.sqrt(DH))

    sq = _stackT(wq) * scale
    wq_t = np.ascontiguousarray(
        sq[:, qcols].reshape(KC_D, 128, MC_D, 128).transpose(2, 1, 0, 3)
    ).astype(NPBF)
    bq_l = (np.concatenate([bq[:, 0], bq[:, 1]]) * scale)[qcols]
    bq_a = np.ascontiguousarray(
        bq_l.reshape(MC_D, 128).T.astype(np.float32)
    )

    sk = _stackT(wk)
    bkst = np.concatenate([bk[:, 0], bk[:, 1]]).astype(np.float32)
    wk_full = sk[:, qcols].copy()           # [2048, 2048]: per head [Kr | Ki]
    bk_l = bkst[qcols].copy()
    for h in range(NH):
        wk_full[:, h * 128 + 64:h * 128 + 128] *= -1.0   # -> [Kr | -Ki]
        bk_l[h * 128 + 64:h * 128 + 128] *= -1.0
    wk_t = np.ascontiguousarray(
        wk_full.reshape(KC_D, 128, MC_D, 128).transpose(2, 1, 0, 3)
    ).astype(NPBF)
    bk_a = np.ascontiguousarray(bk_l.reshape(MC_D, 128).T.astype(np.float32))

    sv = _stackT(wv)
    svq = sv[:, qcols]                       # [2048, 2048]
    wv_t = np.ascontiguousarray(
        svq.reshape(KC_D, 128, NH // 2, 256).transpose(2, 1, 0, 3)
    ).astype(NPBF)
    # obias: V bias deferred through softmax; per head column:
    # rows 0:64 = bvr - bvi (Or), rows 64:128 = bvr + bvi (Oi)
    ob = np.empty((128, NH), dtype=np.float32)
    bvr, bvi = bv[:, 0].astype(np.float32), bv[:, 1].astype(np.float32)
    for h in range(NH):
        sl = slice(h * 64, h * 64 + 64)
        ob[0:64, h] = bvr[sl] - bvi[sl]
        ob[64:128, h] = bvr[sl] + bvi[sl]

    # Gauss c_fc: blocks Wr^T, Wi^T, (Wr+Wi)^T  [1024, 4096]
    fr = w_fc[..., 0].astype(np.float32).T
    fi = w_fc[..., 1].astype(np.float32).T
    wfc_t = np.ascontiguousarray(
        np.stack([fr, fi, fr + fi])
        .reshape(3, 8, 128, OC_H, 128).transpose(0, 3, 2, 1, 4)
    ).astype(NPBF)
    bfc_a = np.ascontiguousarray(
        np.concatenate([b_fc[:, 0], b_fc[:, 1]])
        .reshape(2 * OC_H, 128).T.astype(np.float32)
    )

    # modReLU fold: M = (0.5 Wp Wfc) x2n + 0.5 Wp |h| + cbias with
    # cbias = 0.5 Wp b_fc + b_proj (complex products on host).
    wpr = w_proj[..., 0].astype(np.float32)
    wpi = w_proj[..., 1].astype(np.float32)
    wfr = w_fc[..., 0].astype(np.float32)
    wfi = w_fc[..., 1].astype(np.float32)
    wcr = 0.5 * (wpr @ wfr - wpi @ wfi)
    wci = 0.5 * (wpr @ wfi + wpi @ wfr)
    wcb_t = np.ascontiguousarray(
        _stackT(np.stack([wcr, wci], axis=-1))
        .reshape(KC_D, 128, MC_D, 128).transpose(2, 1, 0, 3)
    ).astype(NPBF)
    wmg_t = np.ascontiguousarray(
        np.concatenate([0.5 * wpr.T, 0.5 * wpi.T], axis=1)
        .reshape(KC_H, 128, MC_D, 128).transpose(2, 1, 0, 3)
    ).astype(NPBF)
    bfr, bfi = b_fc[:, 0].astype(np.float32), b_fc[:, 1].astype(np.float32)
    cbr = 0.5 * (wpr @ bfr - wpi @ bfi) + b_proj[:, 0].astype(np.float32)
    cbi = 0.5 * (wpr @ bfi + wpi @ bfr) + b_proj[:, 1].astype(np.float32)
    cb_a = np.ascontiguousarray(
        np.concatenate([cbr, cbi]).reshape(MC_D, 128).T.astype(np.float32)
    )

    lng_a = np.ascontiguousarray(
        ln_g.astype(np.float32).reshape(3, 2, 8, 128).transpose(3, 0, 1, 2).reshape(128, 48)
    )
    lnb_a = np.ascontiguousarray(
        ln_b.astype(np.float32).reshape(3, 2, 8, 128).transpose(3, 0, 1, 2).reshape(128, 48)
    )
    return {
        "wq": wq_t, "bq": bq_a, "wk": wk_t, "bk": bk_a, "wv": wv_t, "ob": ob,
        "wfc": wfc_t, "bfc": bfc_a, "wcb": wcb_t, "wmg": wmg_t, "cb": cb_a,
        "lng": lng_a, "lnb": lnb_a,
    }


_NC_CACHE = {}


def kernel(**inputs):
    if "nc" not in _NC_CACHE:
        _NC_CACHE["nc"] = _build_nc()
    nc = _NC_CACHE["nc"]

    x = np.asarray(inputs["x"], dtype=np.float32)
    query = np.asarray(inputs["query"], dtype=np.float32)
    shared = _prep_weights(
        np.asarray(inputs["wq"]), np.asarray(inputs["bq"]),
        np.asarray(inputs["wk"]), np.asarray(inputs["bk"]),
        np.asarray(inputs["wv"]), np.asarray(inputs["bv"]),
        np.asarray(inputs["w_fc"]), np.asarray(inputs["b_fc"]),
        np.asarray(inputs["w_proj"]), np.asarray(inputs["b_proj"]),
        np.asarray(inputs["ln_g"]), np.asarray(inputs["ln_b"]),
    )

    in_maps = []
    for b in range(B):
        zq = np.ascontiguousarray(
            np.concatenate([query[b, :, :, 0].T, query[b, :, :, 1].T], axis=0)
            .reshape(KC_D, 128, T).transpose(1, 0, 2)
        ).astype(NPBF)
        zx = np.ascontiguousarray(
            np.concatenate([x[b, :, :, 0].T, x[b, :, :, 1].T], axis=0)
            .reshape(KC_D, 128, T).transpose(1, 0, 2)
        ).astype(NPBF)
        m = {"zq": zq, "zx": zx}
        m.update(shared)
        in_maps.append(m)

    import os
    trace = bool(os.environ.get("KERNEL_TRACE"))
    res = run_bass_kernel_spmd(nc, in_maps, list(range(N_CORES)), trace=trace)
    _NC_CACHE["exec_time_ns"] = res.exec_time_ns
    out = np.empty((B, S, D, 2), dtype=np.float32)
    for b in range(B):
        yb = res.results[b]["y"].transpose(1, 0, 2).reshape(D2, T)
        out[b, :, :, 0] = yb[:D, :].T
        out[b, :, :, 1] = yb[D:, :].T
    return out


if __name__ == "__main__":
    rng = np.random.default_rng(0)
    f = np.float32
    demo = {
        "x": rng.standard_normal((B, S, D, 2), dtype=f),
        "query": rng.standard_normal((B, S, D, 2), dtype=f),
        "wq": rng.standard_normal((D, D, 2), dtype=f) * 0.02,
        "bq": rng.standard_normal((D, 2), dtype=f) * 0.02,
        "wk": rng.standard_normal((D, D, 2), dtype=f) * 0.02,
        "bk": rng.standard_normal((D, 2), dtype=f) * 0.02,
        "wv": rng.standard_normal((D, D, 2), dtype=f) * 0.02,
        "bv": rng.standard_normal((D, 2), dtype=f) * 0.02,
        "w_fc": rng.standard_normal((HID, D, 2), dtype=f) * 0.02,
        "b_fc": rng.standard_normal((HID, 2), dtype=f) * 0.02,
        "w_proj": rng.standard_normal((D, HID, 2), dtype=f) * 0.02,
        "b_proj": rng.standard_normal((D, 2), dtype=f) * 0.02,
        "ln_g": np.ones((3, 2, D), dtype=f),
        "ln_b": np.zeros((3, 2, D), dtype=f),
    }
    out = kernel(**demo)
    print("out shape", out.shape)
